# revision 1
# baseline (speedup 1.0000x reference)
"""FAVOR+ (Performer) causal linear attention on 8 Trainium2 NeuronCores.

Problem: B=2, L=2048, H=8, D=64, M=128 random features, fp32.
Sharding: the 16 (b,h) pairs are data-parallel; each of the 8 cores gets 2
pairs and runs the full feature-map + chunked causal scan for them with no
cross-core communication.

Math per (b,h) pair (C=128 position chunks, 16 chunks):
  q' = exp(c*q @ P^T - |c*q|^2/2 - rowmax) + EPS        (c = d^-1/4)
  k' = exp(c*k @ P^T - |c*k|^2/2 - globalmax) + EPS
  (the reference's ratio=1/sqrt(M) scaling cancels in num/den and is dropped)
  out_t = (sum_{s<=t} q'_t.k'_s * v_s) / (sum_{s<=t} q'_t.k'_s)
computed chunk-wise: intra-chunk via a masked [C,C] score matmul, cross-chunk
via a running KV ([M, D+1] with an appended ones column that carries the
denominator) accumulated in PSUM.

The stabilizers are applied OUTSIDE the exp: exp(dash) is computed unbiased
(max exponent ~22 for randn inputs, far below fp32 overflow), then
x' = exp(dash) * exp(-(diag+stab)) + EPS in one fused DVE tensor_scalar per
chunk, with the per-chunk exp(-(diag+stab)) columns produced in one batched
sub + exp per pair.

Host-side prep is layout-only: transposes / chunk-major rearranges so every
DMA moves >=4KB-contiguous runs, and a baked-in ones column on V that turns
the denominator into column 64 of the numerator matmuls.
"""

import numpy as np
from contextlib import ExitStack

import concourse.bass as bass
import concourse.mybir as mybir
from concourse import tile, masks
from concourse.bass_utils import run_bass_kernel_spmd

B, L, H, D, M = 2, 2048, 8, 64, 128
C = 128
NCH = L // C              # 16 chunks
E = D + 1                 # 65: value dim + denominator column
NCORES = 8
PPC = (B * H) // NCORES   # 2 (b,h) pairs per core
EPS = 1e-6
DN = 1.0 / (64.0 ** 0.25)       # data_normalizer
SQS = float(0.5 ** 0.5 * DN)    # Square(x*SQS) summed = |DN*x|^2/2
F32 = mybir.dt.float32
AX = mybir.AxisListType
OP = mybir.AluOpType
AF = mybir.ActivationFunctionType

_cache = {}


def _emit_k_phase(ctx, tc, pools, consts, p, qT, kT, qldp, kldp, vaugp, out):
    nc = tc.nc
    ident, mask_ut, ones_row, cPT = consts
    (big, small, io, scratch, pp128, pp65, kvps_pool, kvsb_pool) = pools

    cs = lambda c: slice(c * C, (c + 1) * C)
    cs64 = lambda c: slice(c * D, (c + 1) * D)
    cs65 = lambda c: slice(c * E, (c + 1) * E)

    # ---- per-pair bulk loads; K inputs first and split so the K-phase
    # matmuls can start after the first piece arrives ----
    HL = L // 2
    kT_sb = big.tile([D, L], F32, tag="kT")
    nc.sync.dma_start(kT_sb[:, 0:HL], kT[p][:, 0:HL])
    nc.sync.dma_start(kT_sb[:, HL:L], kT[p][:, HL:L])
    kld_sb = big.tile([C, NCH * D], F32, tag="kld")
    nc.sync.dma_start(kld_sb[:], kldp[p])
    vaug = big.tile([C, NCH * E], F32, tag="vaug")
    nc.sync.dma_start(vaug[:], vaugp[p])
    qT_sb = big.tile([D, L], F32, tag="qT")
    nc.sync.dma_start(qT_sb[:, 0:HL], qT[p][:, 0:HL])
    nc.sync.dma_start(qT_sb[:, HL:L], qT[p][:, HL:L])
    qld_sb = big.tile([C, NCH * D], F32, tag="qld")
    nc.sync.dma_start(qld_sb[:], qldp[p])
    out_all = big.tile([C, NCH * D], F32, tag="out_all")

    # ---- Phase K1: exp(k_dash) unbiased, running max, diag ----
    Kp_all = big.tile([128, L], F32, tag="kp")
    KpT_all = big.tile([128, L], F32, tag="kpt")
    kdiag = small.tile([128, NCH], F32, tag="kdiag")
    rmax = small.tile([128, 1], F32, tag="rmax")
    nc.any.memset(rmax[:], -3.0e38)
    for c in range(NCH):
        kdps = pp128.tile([C, M], F32, tag="pp128")
        nc.tensor.matmul(kdps[:], lhsT=kT_sb[:, cs(c)], rhs=cPT[:],
                         start=True, stop=True)
        nc.scalar.activation(Kp_all[:, cs(c)], kdps[:], AF.Exp)
        kmx = small.tile([128, 1], F32, tag="kmx")
        nc.vector.tensor_reduce(kmx[:], Kp_all[:, cs(c)], axis=AX.X, op=OP.max)
        nc.vector.tensor_max(rmax[:], rmax[:], kmx[:])
        scr = scratch.tile([C, D], F32, tag="scr")
        nc.gpsimd.tensor_mul(scr[:], kld_sb[:, cs64(c)], kld_sb[:, cs64(c)])
        nc.vector.tensor_reduce(kdiag[:, c:c + 1], scr[:], axis=AX.X, op=OP.add)

    # global stabilizer -> bcolk_all[:, c] = exp(-(diag_c + stab))
    rmx_ps = pp65.tile([1, 128], F32, tag="pp65")
    nc.tensor.transpose(rmx_ps[:], rmax[:], ident[:])
    gmax = small.tile([1, 1], F32, tag="gmax")
    nc.vector.tensor_reduce(gmax[:], rmx_ps[:], axis=AX.X, op=OP.max)
    rgm = small.tile([1, 1], F32, tag="rgm")
    nc.vector.reciprocal(rgm[:], gmax[:])
    nsb_ps = pp65.tile([128, 1], F32, tag="pp65")
    nc.tensor.matmul(nsb_ps[:], lhsT=ones_row[:], rhs=rgm[:], start=True, stop=True)
    rg = small.tile([128, 1], F32, tag="rg")
    nc.vector.tensor_copy(rg[:], nsb_ps[:])
    endk = small.tile([128, NCH], F32, tag="endk")
    nc.scalar.activation(endk[:], kdiag[:], AF.Exp, scale=-0.0625)
    bcolk = small.tile([128, NCH], F32, tag="bcolk")
    nc.vector.tensor_scalar_mul(bcolk[:], endk[:], rg[:])

    # ---- Phase K2: k' = exp(dash)*bcol + EPS, and its transpose ----
    for c in range(NCH):
        nc.gpsimd.tensor_scalar(out=Kp_all[:, cs(c)], in0=Kp_all[:, cs(c)],
                                scalar1=bcolk[:, c:c + 1], scalar2=EPS,
                                op0=OP.mult, op1=OP.add)
        ktps = pp128.tile([C, C], F32, tag="pp128")
        nc.tensor.transpose(ktps[:], Kp_all[:, cs(c)], ident[:])
        nc.scalar.copy(KpT_all[:, cs(c)], ktps[:])

    return (qT_sb, qld_sb, vaug, out_all, Kp_all, KpT_all)


def _emit_q_scan(ctx, tc, pools, consts, p, state, out):
    nc = tc.nc
    ident, mask_ut, ones_row, cPT = consts
    (big, small, io, scratch, pp128, pp65, kvps_pool, kvsb_pool) = pools
    (qT_sb, qld_sb, vaug, out_all, Kp_all, KpT_all) = state

    cs = lambda c: slice(c * C, (c + 1) * C)
    cs64 = lambda c: slice(c * D, (c + 1) * D)
    cs65 = lambda c: slice(c * E, (c + 1) * E)

    # ---- Phase Q: exp(q_dash) unbiased, rowmax, diag ----
    Qe_all = big.tile([128, L], F32, tag="qe")
    qdiag = small.tile([128, NCH], F32, tag="qdiag")
    emax = small.tile([128, NCH], F32, tag="emax")
    for c in range(NCH):
        qdps = pp128.tile([C, M], F32, tag="pp128")
        nc.tensor.matmul(qdps[:], lhsT=qT_sb[:, cs(c)], rhs=cPT[:],
                         start=True, stop=True)
        nc.scalar.activation(Qe_all[:, cs(c)], qdps[:], AF.Exp)
        # rowmax(exp) = exp(rowmax): reduce the SBUF exp copy instead of PSUM
        nc.vector.tensor_reduce(emax[:, c:c + 1], Qe_all[:, cs(c)], axis=AX.X,
                                op=OP.max)
        scrq = scratch.tile([C, D], F32, tag="scr")
        nc.gpsimd.tensor_mul(scrq[:], qld_sb[:, cs64(c)], qld_sb[:, cs64(c)])
        nc.vector.tensor_reduce(qdiag[:, c:c + 1], scrq[:], axis=AX.X, op=OP.add)
    # bcolq = exp(-diag) / emax
    end_ = small.tile([128, NCH], F32, tag="end")
    nc.scalar.activation(end_[:], qdiag[:], AF.Exp, scale=-0.0625)
    remax = small.tile([128, NCH], F32, tag="remax")
    nc.vector.reciprocal(remax[:], emax[:])
    bcolq = small.tile([128, NCH], F32, tag="bcolq")
    nc.vector.tensor_mul(bcolq[:], end_[:], remax[:])

    # q' = exp(dash)*bcol + EPS, transposed into QpT_all (chunk-independent)
    QpT_all = big.tile([128, L], F32, tag="qpt_all")
    for c in range(NCH):
        nc.gpsimd.tensor_scalar(out=Qe_all[:, cs(c)], in0=Qe_all[:, cs(c)],
                                scalar1=bcolq[:, c:c + 1], scalar2=EPS,
                                op0=OP.mult, op1=OP.add)
        qtps = pp128.tile([M, C], F32, tag="pp128")
        nc.tensor.transpose(qtps[:], Qe_all[:, cs(c)], ident[:])
        nc.scalar.copy(QpT_all[:, cs(c)], qtps[:])
    return state + (QpT_all,)


def _emit_scan(ctx, tc, pools, consts, p, state, out):
    nc = tc.nc
    ident, mask_ut, ones_row, cPT = consts
    (big, small, io, scratch, pp128, pp65, kvps_pool, kvsb_pool) = pools
    (qT_sb, qld_sb, vaug, out_all, Kp_all, KpT_all, QpT_all) = state

    cs = lambda c: slice(c * C, (c + 1) * C)
    cs64 = lambda c: slice(c * D, (c + 1) * D)
    cs65 = lambda c: slice(c * E, (c + 1) * E)

    # ---- causal scan: only the KV chain is serial now ----
    # dual PSUM accumulators (even/odd chunks) halve the serial depth;
    # the inter contribution is QpT @ (KV_even + KV_odd).
    kv_ps = [kvps_pool.tile([M, E], F32, tag="kvps", name=f"kvps_{p}_{i}")
             for i in range(2)]
    kv_sb = [None, None]
    for c in range(NCH):
        QpT = QpT_all[:, cs(c)]
        # S^T[j,i] = sum_m K'[j,m] Q'[i,m], then causal mask (keep j<=i)
        stps = pp128.tile([C, C], F32, tag="pp128")
        nc.tensor.matmul(stps[:], lhsT=KpT_all[:, cs(c)], rhs=QpT[:],
                         start=True, stop=True)
        stm = io.tile([C, C], F32, tag="stm")
        nc.vector.tensor_mul(stm[:], stps[:], mask_ut[:])

        # num_aug[i, 0:64]=attention numerator, [i,64]=denominator
        ops_ = pp65.tile([C, E], F32, tag="pp65")
        inters = [par for par in range(2) if kv_sb[par] is not None]
        nc.tensor.matmul(ops_[:], lhsT=stm[:], rhs=vaug[:, cs65(c)],
                         start=True, stop=(not inters))
        for n, par in enumerate(inters):
            nc.tensor.matmul(ops_[:], lhsT=QpT[:], rhs=kv_sb[par][:],
                             start=False, stop=(n == len(inters) - 1))

        # KV state += K'_c^T V_aug_c  (PSUM accumulation, even/odd banks)
        par = c % 2
        nc.tensor.matmul(kv_ps[par][:], lhsT=Kp_all[:, cs(c)],
                         rhs=vaug[:, cs65(c)],
                         start=(c < 2), stop=(c >= NCH - 2),
                         skip_group_check=True)
        if c < NCH - 1:
            kv_sb[par] = kvsb_pool.tile([M, E], F32, tag="kvsb",
                                         name=f"kvsb_{p}_{c}")
            nc.vector.tensor_copy(kv_sb[par][:], kv_ps[par][:])

        rc = small.tile([C, 1], F32, tag="rc")
        nc.vector.reciprocal(rc[:], ops_[:, D:E])
        nc.vector.tensor_scalar_mul(out_all[:, cs64(c)], ops_[:, 0:D], rc[:])

    nc.sync.dma_start(out[p], out_all[:])


def _kernel(ctx, tc, out, qT, kT, qldp, kldp, vaugp, projT):
    nc = tc.nc
    const = ctx.enter_context(tc.tile_pool(name="const", bufs=1))
    big = ctx.enter_context(tc.tile_pool(name="big", bufs=2))
    small = ctx.enter_context(tc.tile_pool(name="small", bufs=4))
    io = ctx.enter_context(tc.tile_pool(name="io", bufs=3))
    scratch = ctx.enter_context(tc.tile_pool(name="scratch", bufs=2))
    pp128 = ctx.enter_context(tc.tile_pool(name="pp128", bufs=3, space="PSUM"))
    pp65 = ctx.enter_context(tc.tile_pool(name="pp65", bufs=3, space="PSUM"))
    kvps_pool = ctx.enter_context(tc.tile_pool(name="kvps", bufs=2, space="PSUM"))
    kvsb_pool = ctx.enter_context(tc.tile_pool(name="kvsb", bufs=3))

    ident = const.tile([128, 128], F32)
    masks.make_identity(nc, ident[:])
    mask_ut = const.tile([128, 128], F32)
    masks.make_upper_triangular(nc, mask_ut[:], val=1.0, diag=True)
    ones_row = const.tile([1, 128], F32)
    nc.any.memset(ones_row[:], 1.0)
    projT_sb = const.tile([D, M], F32)
    nc.sync.dma_start(projT_sb[:], projT[:])
    cPT = const.tile([D, M], F32)
    nc.vector.tensor_scalar_mul(cPT[:], projT_sb[:], DN)

    pools = (big, small, io, scratch, pp128, pp65, kvps_pool, kvsb_pool)
    consts = (ident, mask_ut, ones_row, cPT)
    states = [
        _emit_k_phase(ctx, tc, pools, consts, p, qT, kT, qldp, kldp, vaugp, out)
        for p in range(PPC)
    ]
    for p in range(PPC):
        st = _emit_q_scan(ctx, tc, pools, consts, p, states[p], out)
        _emit_scan(ctx, tc, pools, consts, p, st, out)


def _split_multiwaits(nc):
    """The installed walrus encodes at most ONE semaphore wait per
    instruction (EventSemaphore excepted, which takes two).  Hoist extra
    wait conditions onto preceding EventSemaphores on the same engine —
    pure wait instructions, no pipeline flush."""
    fix_id = [0]

    def wait_ev(engine, waits):
        fix_id[0] += 1
        return mybir.InstEventSemaphore(
            name=f"I-waitfix-{fix_id[0]}",
            opcode="EventSemaphore",
            engine=engine,
            ins=[], outs=[],
            sync_info=mybir.SyncInfo(on_wait=list(waits), on_update=[]),
        )

    for fn in nc.m.functions:
        for blk in fn.blocks:
            new_insts = []
            for inst in blk.instructions:
                si = inst.sync_info
                waits = list(si.on_wait) if si is not None else []
                is_ev = type(inst).__name__ == "InstEventSemaphore"
                cap = 2 if is_ev else 1
                if len(waits) > cap:
                    extra, keep = waits[:-cap], waits[-cap:]
                    for i in range(0, len(extra), 2):
                        new_insts.append(wait_ev(inst.engine, extra[i:i + 2]))
                    si.on_wait = keep
                new_insts.append(inst)
            blk.instructions[:] = new_insts


def _build():
    if 'nc' in _cache:
        return _cache['nc']
    nc = bass.Bass("TRN2", target_bir_lowering=False, debug=False,
                   num_devices=NCORES)
    qT = nc.dram_tensor("qT", [PPC, D, L], F32, kind="ExternalInput").ap()
    kT = nc.dram_tensor("kT", [PPC, D, L], F32, kind="ExternalInput").ap()
    qldp = nc.dram_tensor("qldp", [PPC, C, NCH * D], F32, kind="ExternalInput").ap()
    kldp = nc.dram_tensor("kldp", [PPC, C, NCH * D], F32, kind="ExternalInput").ap()
    vaugp = nc.dram_tensor("vaugp", [PPC, C, NCH * E], F32, kind="ExternalInput").ap()
    projT = nc.dram_tensor("projT", [D, M], F32, kind="ExternalInput").ap()
    out = nc.dram_tensor("out", [PPC, C, NCH * D], F32, kind="ExternalOutput").ap()
    with tile.TileContext(nc) as tc:
        with ExitStack() as ctx:
            _kernel(ctx, tc, out, qT, kT, qldp, kldp, vaugp, projT)
    _split_multiwaits(nc)
    _cache['nc'] = nc
    return nc


def kernel(query, key, value, projection_matrix, _trace=False):
    """Full inputs in, full output out. Shards (b,h) pairs across 8 cores."""
    query = np.asarray(query, dtype=np.float32)
    key = np.asarray(key, dtype=np.float32)
    value = np.asarray(value, dtype=np.float32)
    projection_matrix = np.ascontiguousarray(
        np.asarray(projection_matrix, dtype=np.float32))

    nc = _build()

    # [B,L,H,D] -> [B*H, L, D] pair-major
    def pairs_ld(x):
        return np.ascontiguousarray(x.transpose(0, 2, 1, 3).reshape(B * H, L, D))

    # chunk-major [B*H, 128, NCH*D]: row p holds [chunk][d] for position p
    def chunkmaj(x_ld):
        return np.ascontiguousarray(
            x_ld.reshape(B * H, NCH, C, D).transpose(0, 2, 1, 3)
            .reshape(B * H, C, NCH * D))

    q_ld = pairs_ld(query)
    k_ld = pairs_ld(key)
    v_ld = pairs_ld(value)
    q_T = np.ascontiguousarray(q_ld.transpose(0, 2, 1))  # [B*H, D, L]
    k_T = np.ascontiguousarray(k_ld.transpose(0, 2, 1))
    q_cm = chunkmaj(q_ld)
    k_cm = chunkmaj(k_ld)
    # V with a baked ones column: [B*H, 128, NCH*(D+1)]
    v4 = v_ld.reshape(B * H, NCH, C, D).transpose(0, 2, 1, 3)  # [P,128,NCH,D]
    vaug = np.concatenate(
        [v4, np.ones((B * H, C, NCH, 1), dtype=np.float32)], axis=3)
    vaug = np.ascontiguousarray(vaug.reshape(B * H, C, NCH * E))

    in_maps = []
    for r in range(NCORES):
        sl = slice(r * PPC, (r + 1) * PPC)
        in_maps.append({
            "qT": q_T[sl], "kT": k_T[sl],
            "qldp": q_cm[sl], "kldp": k_cm[sl], "vaugp": vaug[sl],
            "projT": projection_matrix.T.copy(),
        })

    res = run_bass_kernel_spmd(nc, in_maps, list(range(NCORES)), trace=_trace)
    out_cm = np.empty((B * H, C, NCH * D), dtype=np.float32)
    for r in range(NCORES):
        out_cm[r * PPC:(r + 1) * PPC] = res.results[r]["out"]
    # chunk-major -> [B*H, L, D] -> [B, L, H, D]
    out_ld = out_cm.reshape(B * H, C, NCH, D).transpose(0, 2, 1, 3).reshape(
        B * H, L, D)
    full = out_ld.reshape(B, H, L, D).transpose(0, 2, 1, 3)
    if _trace:
        return np.ascontiguousarray(full), res
    return np.ascontiguousarray(full)



# revision 18
# speedup vs baseline: 1.4862x; 1.4862x over previous
"""FAVOR+ (Performer) causal linear attention on 8 Trainium2 NeuronCores.

Problem: B=2, L=2048, H=8, D=64, M=128 random features, fp32.
Sharding: the 16 (b,h) pairs are data-parallel; each of the 8 cores gets 2
pairs and runs the full feature-map + chunked causal scan for them with no
cross-core communication.

Math per (b,h) pair (C=128 position chunks, 16 chunks), exactly matching the
reference semantics including the +EPS terms (which are NOT negligible here:
typical k' values are within an order of magnitude of EPS):
  q'_t = exp(qdash_t - qdiag_t - qmax_t) + EPS     (per-position stabilizer)
  k'_s = exp(kdash_s - kdiag_s - gmax) + EPS       (global stabilizer)
  out_t = (sum_{s<=t} q'_t.k'_s * v_s) / (sum_{s<=t} q'_t.k'_s)
  (the reference's ratio=1/sqrt(M) cancels in num/den and is dropped)

On-device numerics are fp16 with three exact algebraic rescalings that keep
every intermediate inside fp16 range:
  - exp is computed with a constant bias: Qe = exp(qdash - 7.5). The bias
    cancels because the stabilizer divides by the max of the SAME biased
    exponentials (emax / Gexp).
  - q' is scaled by BETA=64 and k' by ALPHA=1024 (EPS scaled along), global
    per-side constants that cancel in num/den. This lifts the S = k'.q'
    products (~1e-12 in reference units) out of fp16-underflow territory.
Validated vs the fp32 reference on CPU: rel-to-scale err ~1.3e-3.

Structure per pair: 16 per-chunk dash matmuls (fp16, shared moving cPT) into
a 4-bank PSUM strip, ONE batched exp [128,2048] -> fp16, segmented-AP DVE
reduces for diag/max columns [C,16], per-chunk scale+EPS, PE transposes for
the feature-major layouts, then the chunked causal scan: masked [C,C] score
matmul (intra) + running KV state [M,65] in dual-parity PSUM banks (inter),
with the denominator carried as a baked ones column of V.
"""

import numpy as np
from contextlib import ExitStack

import concourse.bass as bass
import concourse.mybir as mybir
from concourse import tile, masks
from concourse.bass_utils import run_bass_kernel_spmd

B, L, H, D, M = 2, 2048, 8, 64, 128
C = 128
NCH = L // C              # 16 chunks
E = D + 1                 # 65: value dim + denominator column
NCORES = 8
PPC = (B * H) // NCORES   # 2 (b,h) pairs per core
EPS = 1e-6
DN = 1.0 / (64.0 ** 0.25)       # data_normalizer c
XBIAS = -7.5                    # constant exp bias (cancels via emax/Gexp)
ALPHA = 1024.0                  # k' global rescale (cancels in num/den)
BETA = 64.0                     # q' global rescale (cancels in num/den)
LN_ALPHA = float(np.log(ALPHA))
LN_BETA = float(np.log(BETA))
F32 = mybir.dt.float32
F16 = mybir.dt.float16
AX = mybir.AxisListType
OP = mybir.AluOpType
AF = mybir.ActivationFunctionType

_cache = {}


def _emit_load(ctx, tc, pools, p, xT, xcm, vaugp):
    nc = tc.nc
    (const, big, small, scr, pdash, ptr, pst, pops, kvps_pool, kvsb_pool,
     psmall) = pools
    kT_sb = big.tile([D, L], F16, tag="kT", name=f"kT_{p}")
    nc.sync.dma_start(kT_sb[:], xT[p, 0])
    kcm_sb = big.tile([C, NCH * D], F16, tag="kcm", name=f"kcm_{p}")
    nc.sync.dma_start(kcm_sb[:], xcm[p, 0])
    vaug_sb = big.tile([C, NCH * E], F16, tag="vaug", name=f"vaug_{p}")
    nc.sync.dma_start(vaug_sb[:], vaugp[p])
    qT_sb = big.tile([D, L], F16, tag="qT", name=f"qT_{p}")
    nc.sync.dma_start(qT_sb[:], xT[p, 1])
    qcm_sb = big.tile([C, NCH * D], F16, tag="qcm", name=f"qcm_{p}")
    nc.sync.dma_start(qcm_sb[:], xcm[p, 1])
    return kT_sb, kcm_sb, vaug_sb, qT_sb, qcm_sb


def _emit_side(ctx, tc, pools, consts, p, side, xT_sb, xcm_sb):
    """Feature map for one side (k or q) of one pair.

    Returns (Xp_all [C, NCH*M] fp16 position-major primed features or None
    for q, XpT_all [M, L] fp16 feature-major primed features)."""
    nc = tc.nc
    identF16, mask_ut, ones_row, cPT, biasx, biasA, biasB = consts
    (const, big, small, scr, pdash, ptr, pst, pops, kvps_pool, kvsb_pool,
     psmall) = pools
    is_k = side == "k"
    cs = lambda c: slice(c * C, (c + 1) * C)

    # dash: per-chunk matmuls into 1-bank PSUM strips (4 chunks each),
    # shared moving cPT; one batched biased exp per strip
    Xe_all = big.tile([C, NCH * M], F16, tag=f"{side}e", name=f"{side}e_{p}")
    for g in range(NCH // 4):
        xd_ps = pdash.tile([C, 4 * C], F32, tag="dash",
                           name=f"dash_{p}{side}{g}")
        for i in range(4):
            c = 4 * g + i
            nc.tensor.matmul(xd_ps[:, i * C:(i + 1) * C],
                             lhsT=xT_sb[:, cs(c)], rhs=cPT[:],
                             start=True, stop=True)
        nc.scalar.activation(Xe_all[:, g * 4 * C:(g + 1) * 4 * C], xd_ps[:],
                             AF.Exp, bias=biasx[:])

    # diag column: square chunk-major x (Pool), segmented add-reduce (DVE)
    xsq = scr.tile([C, NCH * D], F16, tag="xsq", name=f"xsq_{p}{side}")
    nc.gpsimd.tensor_mul(xsq[:], xcm_sb[:], xcm_sb[:])
    xdiag = small.tile([C, NCH], F16, tag="xdiag", name=f"xdiag_{p}{side}")
    with nc.allow_low_precision(reason="sum of 64 fp16 squares; validated "
                                "diag abs err ~2e-3 -> 0.2% weight error"):
        nc.vector.tensor_reduce(xdiag[:],
                                xsq[:].rearrange("p (c d) -> p c d", c=NCH),
                                axis=AX.X, op=OP.add)
    # endx = SCALE * exp(-c^2/2 * sum x^2): the 0.0625 folds c^2/2
    endx = small.tile([C, NCH], F32, tag="endx", name=f"endx_{p}{side}")
    nc.scalar.activation(endx[:], xdiag[:], AF.Exp, scale=-0.0625,
                         bias=(biasA[:] if is_k else biasB[:]))

    bcol = small.tile([C, NCH], F32, tag="bcol", name=f"bcol_{p}{side}")
    if is_k:
        # global stabilizer: 1/max over ALL (position, m) of the biased exp
        gm = small.tile([1, 1], F32, tag="gm", name=f"gm_{p}")
        nc.gpsimd.tensor_reduce(gm[:], Xe_all[:], axis=AX.XYZWC, op=OP.max)
        rg = small.tile([1, 1], F32, tag="rg", name=f"rg_{p}")
        nc.vector.reciprocal(rg[:], gm[:])
        nsb = ptr.tile([C, 1], F32, tag="st", name=f"nsb_{p}")
        nc.tensor.matmul(nsb[:], lhsT=ones_row[:], rhs=rg[:], start=True,
                         stop=True)
        rgb = small.tile([C, 1], F32, tag="rgb", name=f"rgb_{p}")
        nc.vector.tensor_copy(rgb[:], nsb[:])
        nc.vector.tensor_scalar_mul(bcol[:], endx[:], rgb[:])
    else:
        # per-position stabilizer: exp(-diag)/rowmax(exp)
        emax = small.tile([C, NCH], F16, tag="emax", name=f"emax_{p}")
        nc.vector.tensor_reduce(emax[:],
                                Xe_all[:].rearrange("p (c m) -> p c m", c=NCH),
                                axis=AX.X, op=OP.max)
        remax = small.tile([C, NCH], F32, tag="remax", name=f"remax_{p}")
        nc.vector.reciprocal(remax[:], emax[:])
        nc.vector.tensor_mul(bcol[:], endx[:], remax[:])

    # x' = exp * bcol + EPS' (Pool), then PE-transpose each chunk to
    # feature-major; 4 transposed chunks batch into one f16 PSUM tile so a
    # single fp16-2x copy drains them (alternating DVE / Act)
    eps_s = ALPHA * EPS if is_k else BETA * EPS
    XpT_all = big.tile([M, L], F16, tag=f"{side}pT", name=f"{side}pT_{p}")
    Xp_all = big.tile([C, NCH * M], F16, tag=f"{side}p", name=f"{side}p_{p}")
    for g in range(NCH // 4):
        tp4 = pdash.tile([M, 4 * C], F16, tag="dash", name=f"tp4_{p}{side}{g}")
        for i in range(4):
            c = 4 * g + i
            nc.gpsimd.tensor_scalar(out=Xp_all[:, cs(c)],
                                    in0=Xe_all[:, cs(c)],
                                    scalar1=bcol[:, c:c + 1], scalar2=eps_s,
                                    op0=OP.mult, op1=OP.add)
            nc.tensor.transpose(tp4[:, i * C:(i + 1) * C], Xp_all[:, cs(c)],
                                identF16[:])
        dst = XpT_all[:, g * 4 * C:(g + 1) * 4 * C]
        if g % 2 == 0:
            nc.vector.tensor_copy(dst, tp4[:])
        else:
            nc.scalar.activation(dst, tp4[:], AF.Copy)
    return (Xp_all if is_k else None), XpT_all


def _emit_scan(ctx, tc, pools, consts, p, state, vaug_sb, out):
    nc = tc.nc
    identF16, mask_ut, ones_row, cPT, biasx, biasA, biasB = consts
    (const, big, small, scr, pdash, ptr, pst, pops, kvps_pool, kvsb_pool,
     psmall) = pools
    Kp_all, KpT_all, QpT_all = state

    cs = lambda c: slice(c * C, (c + 1) * C)
    cs64 = lambda c: slice(c * D, (c + 1) * D)
    cs65 = lambda c: slice(c * E, (c + 1) * E)

    out_all = big.tile([C, NCH * D], F16, tag="out_all", name=f"out_all_{p}")
    kv_ps = [kvps_pool.tile([M, E], F32, tag="kvps", name=f"kvps_{p}_{i}")
             for i in range(2)]
    kv_sb = [None, None]
    for c in range(NCH):
        # intra-chunk scores S^T[j,i] then causal mask (keep j<=i)
        st_ps = ptr.tile([C, C], F32, tag="st", name=f"st_{p}{c}")
        nc.tensor.matmul(st_ps[:], lhsT=KpT_all[:, cs(c)],
                         rhs=QpT_all[:, cs(c)], start=True, stop=True)
        stm = scr.tile([C, C], F16, tag="stm", name=f"stm_{p}{c}")
        nc.vector.tensor_mul(stm[:], st_ps[:], mask_ut[:])

        # num_aug[i,0:64] numerator, [i,64] denominator
        ops_ps = pops.tile([C, E], F32, tag="ops", name=f"ops_{p}{c}")
        inters = [par for par in range(2) if kv_sb[par] is not None]
        if len(inters) == 2:
            kvc = kvsb_pool.tile([M, E], F16, tag="kvc", name=f"kvc_{p}{c}")
            nc.gpsimd.tensor_add(kvc[:], kv_sb[0][:], kv_sb[1][:])
            rhs_list = [kvc]
        elif len(inters) == 1:
            rhs_list = [kv_sb[inters[0]]]
        else:
            rhs_list = []
        nc.tensor.matmul(ops_ps[:], lhsT=stm[:], rhs=vaug_sb[:, cs65(c)],
                         start=True, stop=(not rhs_list))
        for n, kvt in enumerate(rhs_list):
            nc.tensor.matmul(ops_ps[:], lhsT=QpT_all[:, cs(c)], rhs=kvt[:],
                             start=False, stop=(n == len(rhs_list) - 1))

        # KV state += K'_c^T V_aug_c (dual-parity PSUM accumulation)
        par = c % 2
        nc.tensor.matmul(kv_ps[par][:], lhsT=Kp_all[:, cs(c)],
                         rhs=vaug_sb[:, cs65(c)],
                         start=(c < 2), stop=(c >= NCH - 2),
                         skip_group_check=True)
        if c < NCH - 1:
            kv_sb[par] = kvsb_pool.tile([M, E], F16, tag="kvsb",
                                        name=f"kvsb_{p}_{c}")
            if c % 4 < 2:
                nc.vector.tensor_copy(kv_sb[par][:], kv_ps[par][:])
            else:
                nc.scalar.activation(kv_sb[par][:], kv_ps[par][:], AF.Copy)

        rc = small.tile([C, 1], F32, tag="rc", name=f"rc_{p}{c}")
        nc.vector.reciprocal(rc[:], ops_ps[:, D:E])
        nc.scalar.activation(out_all[:, cs64(c)], ops_ps[:, 0:D], AF.Copy,
                             scale=rc[:])

    nc.sync.dma_start(out[p], out_all[:])


def _kernel(ctx, tc, out, xT, xcm, vaugp, cPTd):
    nc = tc.nc
    const = ctx.enter_context(tc.tile_pool(name="const", bufs=1))
    big = ctx.enter_context(tc.tile_pool(name="big", bufs=2))
    small = ctx.enter_context(tc.tile_pool(name="small", bufs=4))
    scr = ctx.enter_context(tc.tile_pool(name="scr", bufs=3))
    pdash = ctx.enter_context(tc.tile_pool(name="pdash", bufs=2, space="PSUM"))
    ptr = ctx.enter_context(tc.tile_pool(name="ptr", bufs=2, space="PSUM"))
    pops = ctx.enter_context(tc.tile_pool(name="pops", bufs=2, space="PSUM"))
    kvps_pool = ctx.enter_context(tc.tile_pool(name="kvps", bufs=2,
                                               space="PSUM"))
    kvsb_pool = ctx.enter_context(tc.tile_pool(name="kvsb", bufs=4))
    pst = psmall = ptr
    pools = (const, big, small, scr, pdash, ptr, pst, pops, kvps_pool,
             kvsb_pool, psmall)

    identF16 = const.tile([128, 128], F16)
    masks.make_identity(nc, identF16[:])
    mask_ut = const.tile([128, 128], F16)
    masks.make_upper_triangular(nc, mask_ut[:], val=1.0, diag=True)
    ones_row = const.tile([1, 128], F32)
    nc.any.memset(ones_row[:], 1.0)
    cPT = const.tile([D, M], F16)
    nc.sync.dma_start(cPT[:], cPTd[:])
    biasx = const.tile([128, 1], F32)
    nc.any.memset(biasx[:], XBIAS)
    biasA = const.tile([128, 1], F32)
    nc.any.memset(biasA[:], LN_ALPHA)
    biasB = const.tile([128, 1], F32)
    nc.any.memset(biasB[:], LN_BETA)
    consts = (identF16, mask_ut, ones_row, cPT, biasx, biasA, biasB)

    loads = [_emit_load(ctx, tc, pools, p, xT, xcm, vaugp)
             for p in range(PPC)]
    sides = []
    for p in range(PPC):
        kT_sb, kcm_sb, vaug_sb, qT_sb, qcm_sb = loads[p]
        Kp, KpT = _emit_side(ctx, tc, pools, consts, p, "k", kT_sb, kcm_sb)
        sides.append([Kp, KpT])
    for p in range(PPC):
        kT_sb, kcm_sb, vaug_sb, qT_sb, qcm_sb = loads[p]
        _, QpT = _emit_side(ctx, tc, pools, consts, p, "q", qT_sb, qcm_sb)
        sides[p].append(QpT)
    for p in range(PPC):
        _emit_scan(ctx, tc, pools, consts, p, tuple(sides[p]), loads[p][2],
                   out)


def _split_multiwaits(nc):
    """The installed walrus encodes at most ONE semaphore wait per
    instruction (EventSemaphore excepted, which takes two).  Hoist extra
    wait conditions onto preceding EventSemaphores on the same engine —
    pure wait instructions, no pipeline flush."""
    fix_id = [0]

    def wait_ev(engine, waits):
        fix_id[0] += 1
        return mybir.InstEventSemaphore(
            name=f"I-waitfix-{fix_id[0]}",
            opcode="EventSemaphore",
            engine=engine,
            ins=[], outs=[],
            sync_info=mybir.SyncInfo(on_wait=list(waits), on_update=[]),
        )

    for fn in nc.m.functions:
        for blk in fn.blocks:
            new_insts = []
            for inst in blk.instructions:
                si = inst.sync_info
                waits = list(si.on_wait) if si is not None else []
                is_ev = type(inst).__name__ == "InstEventSemaphore"
                cap = 2 if is_ev else 1
                if len(waits) > cap:
                    extra, keep = waits[:-cap], waits[-cap:]
                    for i in range(0, len(extra), 2):
                        new_insts.append(wait_ev(inst.engine, extra[i:i + 2]))
                    si.on_wait = keep
                new_insts.append(inst)
            blk.instructions[:] = new_insts


def _build():
    if 'nc' in _cache:
        return _cache['nc']
    nc = bass.Bass("TRN2", target_bir_lowering=False, debug=False,
                   num_devices=NCORES)
    # xT[p, 0]=kT, xT[p, 1]=qT: [64, L] fp16; xcm likewise chunk-major
    xT = nc.dram_tensor("xT", [PPC, 2, D, L], F16, kind="ExternalInput").ap()
    xcm = nc.dram_tensor("xcm", [PPC, 2, C, NCH * D], F16,
                         kind="ExternalInput").ap()
    vaugp = nc.dram_tensor("vaugp", [PPC, C, NCH * E], F16,
                           kind="ExternalInput").ap()
    cPTd = nc.dram_tensor("cPTd", [D, M], F16, kind="ExternalInput").ap()
    out = nc.dram_tensor("out", [PPC, C, NCH * D], F16,
                         kind="ExternalOutput").ap()
    with tile.TileContext(nc) as tc:
        with ExitStack() as ctx:
            _kernel(ctx, tc, out, xT, xcm, vaugp, cPTd)
    _split_multiwaits(nc)
    _cache['nc'] = nc
    return nc


def kernel(query, key, value, projection_matrix, _trace=False):
    """Full inputs in, full output out. Shards (b,h) pairs across 8 cores."""
    query = np.asarray(query, dtype=np.float32)
    key = np.asarray(key, dtype=np.float32)
    value = np.asarray(value, dtype=np.float32)
    projection_matrix = np.asarray(projection_matrix, dtype=np.float32)

    nc = _build()

    # [B,L,H,D] -> [B*H, L, D] pair-major
    def pairs_ld(x):
        return np.ascontiguousarray(x.transpose(0, 2, 1, 3).reshape(B * H, L, D))

    # chunk-major [B*H, 128, NCH*D]: row p holds [chunk][d] for position p
    def chunkmaj(x_ld):
        return np.ascontiguousarray(
            x_ld.reshape(B * H, NCH, C, D).transpose(0, 2, 1, 3)
            .reshape(B * H, C, NCH * D))

    q_ld = pairs_ld(query)
    k_ld = pairs_ld(key)
    v_ld = pairs_ld(value)
    # stacked [B*H, 2(k,q), 64, L] fp16 transposed inputs
    xT = np.stack([k_ld.transpose(0, 2, 1), q_ld.transpose(0, 2, 1)], axis=1)
    xT = np.ascontiguousarray(xT.astype(np.float16))
    xcm = np.stack([chunkmaj(k_ld), chunkmaj(q_ld)], axis=1)
    xcm = np.ascontiguousarray(xcm.astype(np.float16))
    # V with a baked ones column: [B*H, 128, NCH*(D+1)] fp16
    v4 = v_ld.reshape(B * H, NCH, C, D).transpose(0, 2, 1, 3)
    vaug = np.concatenate(
        [v4, np.ones((B * H, C, NCH, 1), dtype=np.float32)], axis=3)
    vaug = np.ascontiguousarray(
        vaug.reshape(B * H, C, NCH * E).astype(np.float16))
    cPT = np.ascontiguousarray(
        (DN * projection_matrix).T.astype(np.float16))

    in_maps = []
    for r in range(NCORES):
        sl = slice(r * PPC, (r + 1) * PPC)
        in_maps.append({
            "xT": xT[sl], "xcm": xcm[sl], "vaugp": vaug[sl],
            "cPTd": cPT.copy(),
        })

    res = run_bass_kernel_spmd(nc, in_maps, list(range(NCORES)), trace=_trace)
    out_cm = np.empty((B * H, C, NCH * D), dtype=np.float32)
    for r in range(NCORES):
        out_cm[r * PPC:(r + 1) * PPC] = np.asarray(
            res.results[r]["out"], dtype=np.float32)
    # chunk-major -> [B*H, L, D] -> [B, L, H, D]
    out_ld = out_cm.reshape(B * H, C, NCH, D).transpose(0, 2, 1, 3).reshape(
        B * H, L, D)
    full = out_ld.reshape(B, H, L, D).transpose(0, 2, 1, 3)
    if _trace:
        return np.ascontiguousarray(full), res
    return np.ascontiguousarray(full)


# revision 32
# speedup vs baseline: 1.7262x; 1.1616x over previous
"""FAVOR+ (Performer) causal linear attention on 8 Trainium2 NeuronCores.

Problem: B=2, L=2048, H=8, D=64, M=128 random features, fp32.
Sharding: the 16 (b,h) pairs are data-parallel; each of the 8 cores gets 2
pairs and runs the full feature-map + chunked causal scan for them with no
cross-core communication.

Math per (b,h) pair (C=128 position chunks, 16 chunks), exactly matching the
reference semantics including the +EPS terms (which are NOT negligible here:
typical k' values are within an order of magnitude of EPS):
  q'_t = exp(qdash_t - qdiag_t - qmax_t) + EPS     (per-position stabilizer)
  k'_s = exp(kdash_s - kdiag_s - gmax) + EPS       (global stabilizer)
  out_t = (sum_{s<=t} q'_t.k'_s * v_s) / (sum_{s<=t} q'_t.k'_s)
  (the reference's ratio=1/sqrt(M) cancels in num/den and is dropped)

On-device numerics are fp16 with three exact algebraic rescalings that keep
every intermediate inside fp16 range:
  - exp is computed with a constant bias: Qe = exp(qdash - 7.5). The bias
    cancels because the stabilizer divides by the max of the SAME biased
    exponentials (emax / Gexp).
  - q' is scaled by BETA=64 and k' by ALPHA=1024 (EPS scaled along), global
    per-side constants that cancel in num/den. This lifts the S = k'.q'
    products (~1e-12 in reference units) out of fp16-underflow territory.
Validated vs the fp32 reference on CPU: rel-to-scale err ~1.3e-3.

Structure per pair: 16 per-chunk dash matmuls (fp16, shared moving cPT) into
a 4-bank PSUM strip, ONE batched exp [128,2048] -> fp16, segmented-AP DVE
reduces for diag/max columns [C,16], per-chunk scale+EPS, PE transposes for
the feature-major layouts, then the chunked causal scan: masked [C,C] score
matmul (intra) + running KV state [M,65] in dual-parity PSUM banks (inter),
with the denominator carried as a baked ones column of V.
"""

import numpy as np
from contextlib import ExitStack

import concourse.bass as bass
import concourse.mybir as mybir
from concourse import tile, masks
from concourse.bass_utils import run_bass_kernel_spmd

B, L, H, D, M = 2, 2048, 8, 64, 128
C = 128
NCH = L // C              # 16 chunks
E = D + 1                 # 65: value dim + denominator column
NCORES = 8
PPC = (B * H) // NCORES   # 2 (b,h) pairs per core
EPS = 1e-6
DN = 1.0 / (64.0 ** 0.25)       # data_normalizer c
XBIAS = -7.5                    # constant exp bias (cancels via emax/Gexp)
ALPHA = 1024.0                  # k' global rescale (cancels in num/den)
BETA = 64.0                     # q' global rescale (cancels in num/den)
LN_ALPHA = float(np.log(ALPHA))
LN_BETA = float(np.log(BETA))
F32 = mybir.dt.float32
F16 = mybir.dt.float16
AX = mybir.AxisListType
OP = mybir.AluOpType
AF = mybir.ActivationFunctionType

_cache = {}


def _emit_load(ctx, tc, pools, p, xT, xcm, vaugp):
    nc = tc.nc
    (const, big, small, scr, pdash, ptr, pst, pops, kvps_pool, kvsb_pool,
     psmall) = pools
    HL = L // 2
    kT_sb = big.tile([2 * D, HL], F16, tag="kT", name=f"kT_{p}")
    nc.sync.dma_start(kT_sb[:, 0:HL // 2], xT[p, 0][:, 0:HL // 2])
    nc.sync.dma_start(kT_sb[:, HL // 2:HL], xT[p, 0][:, HL // 2:HL])
    kcm_sb = big.tile([C, NCH * D], F16, tag="kcm", name=f"kcm_{p}")
    nc.sync.dma_start(kcm_sb[:], xcm[p, 0])
    qT_sb = big.tile([2 * D, HL], F16, tag="qT", name=f"qT_{p}")
    qcm_sb = big.tile([C, NCH * D], F16, tag="qcm", name=f"qcm_{p}")
    vaug_sb = big.tile([C, NCH * E], F16, tag="vaug", name=f"vaug_{p}")
    return kT_sb, kcm_sb, vaug_sb, qT_sb, qcm_sb


def _emit_load2(ctx, tc, loads, p, xT, xcm, vaugp):
    nc = tc.nc
    kT_sb, kcm_sb, vaug_sb, qT_sb, qcm_sb = loads[p]
    nc.sync.dma_start(qT_sb[:], xT[p, 1])
    nc.sync.dma_start(qcm_sb[:], xcm[p, 1])
    nc.sync.dma_start(vaug_sb[:], vaugp[p])


def _emit_side(ctx, tc, pools, consts, p, side, xT_sb, xcm_sb):
    """Feature map for one side (k or q) of one pair.

    Returns (Xp_all [C, NCH*M] fp16 position-major primed features or None
    for q, XpT_all [M, L] fp16 feature-major primed features)."""
    nc = tc.nc
    identF16, mask_ut, ones_row, cPT, biasx, biasA, biasB = consts
    (const, big, small, scr, pdash, ptr, pst, pops, kvps_pool, kvsb_pool,
     psmall) = pools
    is_k = side == "k"
    cs = lambda c: slice(c * C, (c + 1) * C)

    # dash: per-chunk matmuls into 1-bank PSUM strips (4 chunks each),
    # shared moving cPT; one batched biased exp per strip
    Xe_all = big.tile([C, NCH * M], F16, tag=f"{side}e", name=f"{side}e_{p}")
    if is_k:
        gmp = small.tile([1, NCH // 4], F32, tag="gmp", name=f"gmp_{p}")
    else:
        emax = small.tile([C, NCH], F16, tag="emax", name=f"emax_{p}")
    for g in range(NCH // 4):
        xd_ps = pdash.tile([C, 4 * C], F32, tag="dash",
                           name=f"dash_{p}{side}{g}")
        for i in range(4):
            c = 4 * g + i
            if c < NCH // 2:
                lhsT, rhs = xT_sb[0:D, cs(c)], cPT[0:D, :]
            else:
                lhsT, rhs = xT_sb[D:2 * D, cs(c - NCH // 2)], cPT[D:2 * D, :]
            nc.tensor.matmul(xd_ps[:, i * C:(i + 1) * C],
                             lhsT=lhsT, rhs=rhs,
                             start=True, stop=True)
        strip = Xe_all[:, g * 4 * C:(g + 1) * 4 * C]
        nc.scalar.activation(strip, xd_ps[:], AF.Exp, bias=biasx[:])
        # per-strip stabilizer reduce, pipelined behind the next strip
        if is_k:
            nc.gpsimd.tensor_reduce(gmp[:, g:g + 1], strip,
                                    axis=AX.XYZWC, op=OP.max)
        else:
            nc.vector.tensor_reduce(
                emax[:, 4 * g:4 * (g + 1)],
                strip.rearrange("p (c m) -> p c m", c=4),
                axis=AX.X, op=OP.max)

    # diag column: square chunk-major x (Pool), segmented add-reduce (DVE)
    xsq = scr.tile([C, NCH * D], F16, tag="xsq", name=f"xsq_{p}{side}")
    if is_k:
        nc.gpsimd.tensor_mul(xsq[:], xcm_sb[:], xcm_sb[:])
    else:
        nc.vector.tensor_mul(xsq[:], xcm_sb[:], xcm_sb[:])
    xdiag = small.tile([C, NCH], F16, tag="xdiag", name=f"xdiag_{p}{side}")
    with nc.allow_low_precision(reason="sum of 64 fp16 squares; validated "
                                "diag abs err ~2e-3 -> 0.2% weight error"):
        nc.vector.tensor_reduce(xdiag[:],
                                xsq[:].rearrange("p (c d) -> p c d", c=NCH),
                                axis=AX.X, op=OP.add)
    # endx = SCALE * exp(-c^2/2 * sum x^2): the 0.0625 folds c^2/2
    endx = small.tile([C, NCH], F32, tag="endx", name=f"endx_{p}{side}")
    nc.scalar.activation(endx[:], xdiag[:], AF.Exp, scale=-0.0625,
                         bias=(biasA[:] if is_k else biasB[:]))

    bcol = small.tile([C, NCH], F32, tag="bcol", name=f"bcol_{p}{side}")
    if is_k:
        # global stabilizer: 1/max over ALL (position, m) of the biased exp
        gm = small.tile([1, 1], F32, tag="gm", name=f"gm_{p}")
        nc.vector.tensor_reduce(gm[:], gmp[:], axis=AX.X, op=OP.max)
        rg = small.tile([1, 1], F32, tag="rg", name=f"rg_{p}")
        nc.vector.reciprocal(rg[:], gm[:])
        nsb = ptr.tile([C, 1], F32, tag="st", name=f"nsb_{p}")
        nc.tensor.matmul(nsb[:], lhsT=ones_row[:], rhs=rg[:], start=True,
                         stop=True)
        rgb = small.tile([C, 1], F32, tag="rgb", name=f"rgb_{p}")
        nc.vector.tensor_copy(rgb[:], nsb[:])
        nc.vector.tensor_scalar_mul(bcol[:], endx[:], rgb[:])
    else:
        # per-position stabilizer: exp(-diag)/rowmax(exp)
        remax = small.tile([C, NCH], F32, tag="remax", name=f"remax_{p}")
        nc.vector.reciprocal(remax[:], emax[:])
        nc.vector.tensor_mul(bcol[:], endx[:], remax[:])

    # x' = exp * bcol + EPS' (Pool), then PE-transpose each chunk to
    # feature-major; 4 transposed chunks batch into one f16 PSUM tile so a
    # single fp16-2x copy drains them (alternating DVE / Act)
    eps_s = ALPHA * EPS if is_k else BETA * EPS
    XpT_all = big.tile([M, L], F16, tag=f"{side}pT", name=f"{side}pT_{p}")
    Xp_all = big.tile([C, NCH * M], F16, tag=f"{side}p", name=f"{side}p_{p}")
    for g in range(NCH // 4):
        tp4 = pdash.tile([M, 4 * C], F16, tag="dash", name=f"tp4_{p}{side}{g}")
        for i in range(4):
            c = 4 * g + i
            eng = nc.gpsimd if c % 2 == 0 else nc.vector
            eng.tensor_scalar(out=Xp_all[:, cs(c)],
                              in0=Xe_all[:, cs(c)],
                              scalar1=bcol[:, c:c + 1], scalar2=eps_s,
                              op0=OP.mult, op1=OP.add)
            nc.tensor.transpose(tp4[:, i * C:(i + 1) * C], Xp_all[:, cs(c)],
                                identF16[:])
        dst = XpT_all[:, g * 4 * C:(g + 1) * 4 * C]
        if g % 2 == 0:
            nc.vector.tensor_copy(dst, tp4[:])
        else:
            nc.scalar.activation(dst, tp4[:], AF.Copy)
    return (Xp_all if is_k else None), XpT_all


def _emit_scan(ctx, tc, pools, consts, p, state, vaug_sb, out):
    nc = tc.nc
    identF16, mask_ut, ones_row, cPT, biasx, biasA, biasB = consts
    (const, big, small, scr, pdash, ptr, pst, pops, kvps_pool, kvsb_pool,
     psmall) = pools
    Kp_all, KpT_all, QpT_all = state

    cs = lambda c: slice(c * C, (c + 1) * C)
    cs65 = lambda c: slice(c * E, (c + 1) * E)

    out_all = big.tile([C, NCH * D], F16, tag="out_all", name=f"out_all_{p}")
    kv_ps = [kvps_pool.tile([M, E], F32, tag="kvps", name=f"kvps_{p}_{i}")
             for i in range(2)]
    kv_sb = [None, None]
    cs64 = lambda c: slice(c * D, (c + 1) * D)
    for c in range(NCH):
        st_ps = ptr.tile([C, C], F32, tag="st", name=f"st_{p}{c}")
        nc.tensor.matmul(st_ps[:], lhsT=KpT_all[:, cs(c)],
                         rhs=QpT_all[:, cs(c)], start=True, stop=True)
        stm = scr.tile([C, C], F16, tag="stm", name=f"stm_{p}{c}")
        nc.vector.tensor_mul(stm[:], st_ps[:], mask_ut[:])

        ops_ps = pops.tile([C, E], F32, tag="ops", name=f"ops_{p}{c}")
        rhs_list = [kv_sb[par] for par in range(2) if kv_sb[par] is not None]
        nc.tensor.matmul(ops_ps[:], lhsT=stm[:], rhs=vaug_sb[:, cs65(c)],
                         start=True, stop=(not rhs_list))
        for n, kvt in enumerate(rhs_list):
            nc.tensor.matmul(ops_ps[:], lhsT=QpT_all[:, cs(c)], rhs=kvt[:],
                             start=False, stop=(n == len(rhs_list) - 1))

        par = c % 2
        nc.tensor.matmul(kv_ps[par][:],
                         lhsT=Kp_all[:, cs(c)], rhs=vaug_sb[:, cs65(c)],
                         start=(c < 2), stop=(c >= NCH - 2),
                         skip_group_check=True)
        if c < NCH - 1:
            kv_sb[par] = kvsb_pool.tile([M, E], F16, tag="kvsb",
                                        name=f"kvsb_{p}_{c}")
            if c % 4 < 2:
                nc.vector.tensor_copy(kv_sb[par][:], kv_ps[par][:])
            else:
                nc.scalar.activation(kv_sb[par][:], kv_ps[par][:], AF.Copy)

        rc = small.tile([C, 1], F32, tag="rc", name=f"rc_{p}{c}")
        nc.vector.reciprocal(rc[:], ops_ps[:, D:E])
        nc.scalar.activation(out_all[:, cs64(c)], ops_ps[:, 0:D], AF.Copy,
                             scale=rc[:])

    nc.sync.dma_start(out[p], out_all[:])


def _kernel(ctx, tc, out, xT, xcm, vaugp, cPTd):
    nc = tc.nc
    const = ctx.enter_context(tc.tile_pool(name="const", bufs=1))
    big = ctx.enter_context(tc.tile_pool(name="big", bufs=2))
    small = ctx.enter_context(tc.tile_pool(name="small", bufs=8))
    scr = ctx.enter_context(tc.tile_pool(name="scr", bufs=6))
    pdash = ctx.enter_context(tc.tile_pool(name="pdash", bufs=2, space="PSUM"))
    ptr = ctx.enter_context(tc.tile_pool(name="ptr", bufs=2, space="PSUM"))
    pops = ctx.enter_context(tc.tile_pool(name="pops", bufs=2, space="PSUM"))
    kvps_pool = ctx.enter_context(tc.tile_pool(name="kvps", bufs=2,
                                               space="PSUM"))
    kvsb_pool = ctx.enter_context(tc.tile_pool(name="kvsb", bufs=8))
    pst = psmall = ptr
    pools = (const, big, small, scr, pdash, ptr, pst, pops, kvps_pool,
             kvsb_pool, psmall)

    identF16 = const.tile([128, 128], F16)
    masks.make_identity(nc, identF16[:])
    mask_ut = const.tile([128, 128], F16)
    masks.make_upper_triangular(nc, mask_ut[:], val=1.0, diag=True)
    ones_row = const.tile([1, 128], F32)
    nc.any.memset(ones_row[:], 1.0)
    cPT = const.tile([2 * D, M], F16)
    nc.sync.dma_start(cPT[:], cPTd[:])
    biasx = const.tile([128, 1], F32)
    nc.any.memset(biasx[:], XBIAS)
    biasA = const.tile([128, 1], F32)
    nc.any.memset(biasA[:], LN_ALPHA)
    biasB = const.tile([128, 1], F32)
    nc.any.memset(biasB[:], LN_BETA)
    consts = (identF16, mask_ut, ones_row, cPT, biasx, biasA, biasB)

    loads = []
    for p in range(PPC):
        loads.append(_emit_load(ctx, tc, pools, p, xT, xcm, vaugp))
        _emit_load2(ctx, tc, loads, p, xT, xcm, vaugp)
    for p in range(PPC):
        kT_sb, kcm_sb, vaug_sb, qT_sb, qcm_sb = loads[p]
        Kp, KpT = _emit_side(ctx, tc, pools, consts, p, "k", kT_sb, kcm_sb)
        _, QpT = _emit_side(ctx, tc, pools, consts, p, "q", qT_sb, qcm_sb)
        _emit_scan(ctx, tc, pools, consts, p, (Kp, KpT, QpT), vaug_sb, out)


def _split_multiwaits(nc):
    """The installed walrus encodes at most ONE semaphore wait per
    instruction (EventSemaphore excepted, which takes two).  Hoist extra
    wait conditions onto preceding EventSemaphores on the same engine —
    pure wait instructions, no pipeline flush."""
    fix_id = [0]

    def wait_ev(engine, waits):
        fix_id[0] += 1
        return mybir.InstEventSemaphore(
            name=f"I-waitfix-{fix_id[0]}",
            opcode="EventSemaphore",
            engine=engine,
            ins=[], outs=[],
            sync_info=mybir.SyncInfo(on_wait=list(waits), on_update=[]),
        )

    for fn in nc.m.functions:
        for blk in fn.blocks:
            new_insts = []
            for inst in blk.instructions:
                si = inst.sync_info
                waits = list(si.on_wait) if si is not None else []
                is_ev = type(inst).__name__ == "InstEventSemaphore"
                cap = 2 if is_ev else 1
                if len(waits) > cap:
                    extra, keep = waits[:-cap], waits[-cap:]
                    for i in range(0, len(extra), 2):
                        new_insts.append(wait_ev(inst.engine, extra[i:i + 2]))
                    si.on_wait = keep
                new_insts.append(inst)
            blk.instructions[:] = new_insts


def _build():
    if 'nc' in _cache:
        return _cache['nc']
    nc = bass.Bass("TRN2", target_bir_lowering=False, debug=False,
                   num_devices=NCORES)
    # xT[p, 0]=kT, xT[p, 1]=qT: [64, L] fp16; xcm likewise chunk-major
    xT = nc.dram_tensor("xT", [PPC, 2, 2 * D, L // 2], F16,
                        kind="ExternalInput").ap()
    xcm = nc.dram_tensor("xcm", [PPC, 2, C, NCH * D], F16,
                         kind="ExternalInput").ap()
    vaugp = nc.dram_tensor("vaugp", [PPC, C, NCH * E], F16,
                           kind="ExternalInput").ap()
    cPTd = nc.dram_tensor("cPTd", [2 * D, M], F16, kind="ExternalInput").ap()
    out = nc.dram_tensor("out", [PPC, C, NCH * D], F16,
                         kind="ExternalOutput").ap()
    with tile.TileContext(nc) as tc:
        with ExitStack() as ctx:
            _kernel(ctx, tc, out, xT, xcm, vaugp, cPTd)
    _split_multiwaits(nc)
    _cache['nc'] = nc
    return nc


def kernel(query, key, value, projection_matrix, _trace=False):
    """Full inputs in, full output out. Shards (b,h) pairs across 8 cores."""
    query = np.asarray(query, dtype=np.float32)
    key = np.asarray(key, dtype=np.float32)
    value = np.asarray(value, dtype=np.float32)
    projection_matrix = np.asarray(projection_matrix, dtype=np.float32)

    nc = _build()

    # [B,L,H,D] -> [B*H, L, D] pair-major
    def pairs_ld(x):
        return np.ascontiguousarray(x.transpose(0, 2, 1, 3).reshape(B * H, L, D))

    # chunk-major [B*H, 128, NCH*D]: row p holds [chunk][d] for position p
    def chunkmaj(x_ld):
        return np.ascontiguousarray(
            x_ld.reshape(B * H, NCH, C, D).transpose(0, 2, 1, 3)
            .reshape(B * H, C, NCH * D))

    q_ld = pairs_ld(query)
    k_ld = pairs_ld(key)
    v_ld = pairs_ld(value)
    # stacked [B*H, 2(k,q), 128, L/2] fp16: partitions 0-63 d x first
    # L-half, 64-127 d x second L-half (halves per-partition DMA bytes)
    xT = np.stack([k_ld.transpose(0, 2, 1), q_ld.transpose(0, 2, 1)], axis=1)
    xT = xT.reshape(B * H, 2, D, 2, L // 2).transpose(0, 1, 3, 2, 4)
    xT = np.ascontiguousarray(
        xT.reshape(B * H, 2, 2 * D, L // 2).astype(np.float16))
    xcm = np.stack([chunkmaj(k_ld), chunkmaj(q_ld)], axis=1)
    xcm = np.ascontiguousarray(xcm.astype(np.float16))
    # V with a baked ones column: [B*H, 128, NCH*(D+1)] fp16
    v4 = v_ld.reshape(B * H, NCH, C, D).transpose(0, 2, 1, 3)
    vaug = np.concatenate(
        [v4, np.ones((B * H, C, NCH, 1), dtype=np.float32)], axis=3)
    vaug = np.ascontiguousarray(
        vaug.reshape(B * H, C, NCH * E).astype(np.float16))
    cPT1 = (DN * projection_matrix).T.astype(np.float16)
    cPT = np.ascontiguousarray(np.concatenate([cPT1, cPT1], axis=0))

    in_maps = []
    for r in range(NCORES):
        sl = slice(r * PPC, (r + 1) * PPC)
        in_maps.append({
            "xT": xT[sl], "xcm": xcm[sl], "vaugp": vaug[sl],
            "cPTd": cPT.copy(),
        })

    res = run_bass_kernel_spmd(nc, in_maps, list(range(NCORES)), trace=_trace)
    out_cm = np.empty((B * H, C, NCH * D), dtype=np.float32)
    for r in range(NCORES):
        out_cm[r * PPC:(r + 1) * PPC] = np.asarray(
            res.results[r]["out"], dtype=np.float32)
    # chunk-major -> [B*H, L, D] -> [B, L, H, D]
    out_ld = out_cm.reshape(B * H, C, NCH, D).transpose(0, 2, 1, 3).reshape(
        B * H, L, D)
    full = out_ld.reshape(B, H, L, D).transpose(0, 2, 1, 3)
    if _trace:
        return np.ascontiguousarray(full), res
    return np.ascontiguousarray(full)


# revision 39
# speedup vs baseline: 1.7987x; 1.0420x over previous
"""FAVOR+ (Performer) causal linear attention on 8 Trainium2 NeuronCores.

Problem: B=2, L=2048, H=8, D=64, M=128 random features, fp32 in/out.
Sharding: the 16 (b,h) pairs are data-parallel; each of the 8 cores gets 2
pairs and runs the full feature-map + chunked causal scan for them with no
cross-core communication.

Math per (b,h) pair (C=128 position chunks, 16 chunks), exactly matching the
reference semantics including the +EPS terms (which are NOT negligible here:
typical k' values are within an order of magnitude of EPS=1e-6):
  q'_t = exp(qdash_t - qdiag_t - qmax_t) + EPS     (per-position stabilizer)
  k'_s = exp(kdash_s - kdiag_s - gmax) + EPS       (global stabilizer)
  out_t = (sum_{s<=t} q'_t.k'_s * v_s) / (sum_{s<=t} q'_t.k'_s)
  (the reference's ratio=1/sqrt(M) cancels in num/den and is dropped)

On-device numerics are fp16 (1 cyc/row matmuls, fp16-2x DVE modes, half the
DMA bytes) with three exact algebraic rescalings that keep every
intermediate inside fp16 range:
  - exp carries a constant bias: Xe = exp(dash - 7.5). It cancels because
    the stabilizers divide by the max of the SAME biased exponentials.
  - q' is scaled by BETA=64 and k' by ALPHA=1024 (EPS scaled along): global
    per-side constants that cancel in num/den and lift the S = k'.q'
    products (~1e-12 in reference units) out of fp16-underflow territory.
Validated vs the fp32 reference: rel-to-scale err ~1.4e-3 (gate 2e-2).

Engine layout (all PSUM-touching elementwise on DVE/Act only -- the BIR
verifier rejects GPSIMD-PSUM access):
  PE:   16 per-chunk dash matmuls per side (shared cPT, fp16) into 1-bank
        PSUM strips; per-chunk fp16 transposes batched 4-to-a-bank; scan
        matmuls (masked S^T, S~V+2 inter, dual-parity KV accumulation).
  Act:  batched exp per [C,512] strip with AP bias, exp(-diag) columns,
        half the transpose drains, out = num*recip(den) via Copy+scale.
  DVE:  segmented-AP reduces (diag sums, per-strip q rowmax), mask multiply
        (PSUM->SBUF fp16), KV snapshots, reciprocals, half of scale+EPS.
  Pool: squares, scale+EPS, per-strip global-max pieces (SBUF only).
  The k/q stabilizer maxes are computed per exp-strip so they pipeline
  behind the remaining dash matmuls instead of serializing after them.
  xT is shipped as [128, L/2] (two d-halves stacked on partitions) to halve
  per-partition DMA bytes; input DMAs are ordered pair-0-first so the
  pair-0 scan overlaps pair-1 prep.
"""

import numpy as np
from contextlib import ExitStack

import concourse.bass as bass
import concourse.mybir as mybir
from concourse import tile, masks
from concourse.bass_utils import run_bass_kernel_spmd

B, L, H, D, M = 2, 2048, 8, 64, 128
C = 128
NCH = L // C              # 16 chunks
E = D + 1                 # 65: value dim + denominator column
NCORES = 8
PPC = (B * H) // NCORES   # 2 (b,h) pairs per core
EPS = 1e-6
DN = 1.0 / (64.0 ** 0.25)       # data_normalizer c
XBIAS = -7.5                    # constant exp bias (cancels via emax/Gexp)
ALPHA = 1024.0                  # k' global rescale (cancels in num/den)
BETA = 64.0                     # q' global rescale (cancels in num/den)
LN_ALPHA = float(np.log(ALPHA))
LN_BETA = float(np.log(BETA))
F32 = mybir.dt.float32
F16 = mybir.dt.float16
AX = mybir.AxisListType
OP = mybir.AluOpType
AF = mybir.ActivationFunctionType

_cache = {}


def _emit_load(ctx, tc, pools, p, xT, xcm, vaugp):
    nc = tc.nc
    (const, big, small, scr, pdash, ptr, pst, pops, kvps_pool, kvsb_pool,
     psmall) = pools
    HL = L // 2
    kT_sb = big.tile([2 * D, HL], F16, tag="kT", name=f"kT_{p}")
    nc.sync.dma_start(kT_sb[:, 0:HL // 2], xT[p, 0][:, 0:HL // 2])
    nc.sync.dma_start(kT_sb[:, HL // 2:HL], xT[p, 0][:, HL // 2:HL])
    kcm_sb = big.tile([C, NCH * D], F16, tag="kcm", name=f"kcm_{p}")
    nc.sync.dma_start(kcm_sb[:], xcm[p, 0])
    qT_sb = big.tile([2 * D, HL], F16, tag="qT", name=f"qT_{p}")
    qcm_sb = big.tile([C, NCH * D], F16, tag="qcm", name=f"qcm_{p}")
    vaug_sb = big.tile([C, NCH * E], F16, tag="vaug", name=f"vaug_{p}")
    return kT_sb, kcm_sb, vaug_sb, qT_sb, qcm_sb


def _emit_load2(ctx, tc, loads, p, xT, xcm, vaugp):
    nc = tc.nc
    kT_sb, kcm_sb, vaug_sb, qT_sb, qcm_sb = loads[p]
    nc.sync.dma_start(qT_sb[:], xT[p, 1])
    nc.sync.dma_start(qcm_sb[:], xcm[p, 1])
    nc.sync.dma_start(vaug_sb[:], vaugp[p])


def _emit_side(ctx, tc, pools, consts, p, side, xT_sb, xcm_sb):
    """Feature map for one side (k or q) of one pair.

    Returns (Xp_all [C, NCH*M] fp16 position-major primed features or None
    for q, XpT_all [M, L] fp16 feature-major primed features)."""
    nc = tc.nc
    identF16, mask_ut, ones_row, cPT, biasx, biasA, biasB = consts
    (const, big, small, scr, pdash, ptr, pst, pops, kvps_pool, kvsb_pool,
     psmall) = pools
    is_k = side == "k"
    cs = lambda c: slice(c * C, (c + 1) * C)

    # dash: per-chunk matmuls into 1-bank PSUM strips (4 chunks each),
    # shared moving cPT; one batched biased exp per strip
    Xe_all = big.tile([C, NCH * M], F16, tag=f"{side}e", name=f"{side}e_{p}")
    if is_k:
        gmp = small.tile([1, NCH // 4], F32, tag="gmp", name=f"gmp_{p}")
    else:
        emax = small.tile([C, NCH], F16, tag="emax", name=f"emax_{p}")
    for g in range(NCH // 4):
        xd_ps = pdash.tile([C, 4 * C], F32, tag="dash",
                           name=f"dash_{p}{side}{g}")
        for i in range(4):
            c = 4 * g + i
            if c < NCH // 2:
                lhsT, rhs = xT_sb[0:D, cs(c)], cPT[0:D, :]
            else:
                lhsT, rhs = xT_sb[D:2 * D, cs(c - NCH // 2)], cPT[D:2 * D, :]
            nc.tensor.matmul(xd_ps[:, i * C:(i + 1) * C],
                             lhsT=lhsT, rhs=rhs,
                             start=True, stop=True)
        strip = Xe_all[:, g * 4 * C:(g + 1) * 4 * C]
        nc.scalar.activation(strip, xd_ps[:], AF.Exp, bias=biasx[:])
        # per-strip stabilizer reduce, pipelined behind the next strip
        if is_k:
            nc.gpsimd.tensor_reduce(gmp[:, g:g + 1], strip,
                                    axis=AX.XYZWC, op=OP.max)
        else:
            nc.vector.tensor_reduce(
                emax[:, 4 * g:4 * (g + 1)],
                strip.rearrange("p (c m) -> p c m", c=4),
                axis=AX.X, op=OP.max)

    # diag column: square chunk-major x (Pool), segmented add-reduce (DVE)
    xsq = scr.tile([C, NCH * D], F16, tag="xsq", name=f"xsq_{p}{side}")
    if is_k:
        nc.gpsimd.tensor_mul(xsq[:], xcm_sb[:], xcm_sb[:])
    else:
        nc.vector.tensor_mul(xsq[:], xcm_sb[:], xcm_sb[:])
    xdiag = small.tile([C, NCH], F16, tag="xdiag", name=f"xdiag_{p}{side}")
    with nc.allow_low_precision(reason="sum of 64 fp16 squares; validated "
                                "diag abs err ~2e-3 -> 0.2% weight error"):
        nc.vector.tensor_reduce(xdiag[:],
                                xsq[:].rearrange("p (c d) -> p c d", c=NCH),
                                axis=AX.X, op=OP.add)
    # endx = SCALE * exp(-c^2/2 * sum x^2): the 0.0625 folds c^2/2
    endx = small.tile([C, NCH], F32, tag="endx", name=f"endx_{p}{side}")
    nc.scalar.activation(endx[:], xdiag[:], AF.Exp, scale=-0.0625,
                         bias=(biasA[:] if is_k else biasB[:]))

    bcol = small.tile([C, NCH], F32, tag="bcol", name=f"bcol_{p}{side}")
    if is_k:
        # global stabilizer: 1/max over ALL (position, m) of the biased exp
        gm = small.tile([1, 1], F32, tag="gm", name=f"gm_{p}")
        nc.vector.tensor_reduce(gm[:], gmp[:], axis=AX.X, op=OP.max)
        rg = small.tile([1, 1], F32, tag="rg", name=f"rg_{p}")
        nc.vector.reciprocal(rg[:], gm[:])
        nsb = ptr.tile([C, 1], F32, tag="st", name=f"nsb_{p}")
        nc.tensor.matmul(nsb[:], lhsT=ones_row[:], rhs=rg[:], start=True,
                         stop=True)
        rgb = small.tile([C, 1], F32, tag="rgb", name=f"rgb_{p}")
        nc.vector.tensor_copy(rgb[:], nsb[:])
        nc.vector.tensor_scalar_mul(bcol[:], endx[:], rgb[:])
    else:
        # per-position stabilizer: exp(-diag)/rowmax(exp)
        remax = small.tile([C, NCH], F32, tag="remax", name=f"remax_{p}")
        nc.vector.reciprocal(remax[:], emax[:])
        nc.vector.tensor_mul(bcol[:], endx[:], remax[:])

    # x' = exp * bcol + EPS' (Pool), then PE-transpose each chunk to
    # feature-major; 4 transposed chunks batch into one f16 PSUM tile so a
    # single fp16-2x copy drains them (alternating DVE / Act)
    eps_s = ALPHA * EPS if is_k else BETA * EPS
    XpT_all = big.tile([M, L], F16, tag=f"{side}pT", name=f"{side}pT_{p}")
    Xp_all = big.tile([C, NCH * M], F16, tag=f"{side}p", name=f"{side}p_{p}")
    for g in range(NCH // 4):
        tp4 = pdash.tile([M, 4 * C], F16, tag="dash", name=f"tp4_{p}{side}{g}")
        for i in range(4):
            c = 4 * g + i
            eng = nc.gpsimd if c % 2 == 0 else nc.vector
            eng.tensor_scalar(out=Xp_all[:, cs(c)],
                              in0=Xe_all[:, cs(c)],
                              scalar1=bcol[:, c:c + 1], scalar2=eps_s,
                              op0=OP.mult, op1=OP.add)
            nc.tensor.transpose(tp4[:, i * C:(i + 1) * C], Xp_all[:, cs(c)],
                                identF16[:])
        dst = XpT_all[:, g * 4 * C:(g + 1) * 4 * C]
        if g % 2 == 0:
            nc.vector.tensor_copy(dst, tp4[:])
        else:
            nc.scalar.activation(dst, tp4[:], AF.Copy)
    return (Xp_all if is_k else None), XpT_all


def _emit_scan(ctx, tc, pools, consts, p, state, vaug_sb, out):
    nc = tc.nc
    identF16, mask_ut, ones_row, cPT, biasx, biasA, biasB = consts
    (const, big, small, scr, pdash, ptr, pst, pops, kvps_pool, kvsb_pool,
     psmall) = pools
    Kp_all, KpT_all, QpT_all = state

    cs = lambda c: slice(c * C, (c + 1) * C)
    cs65 = lambda c: slice(c * E, (c + 1) * E)

    out_all = big.tile([C, NCH * D], F16, tag="out_all", name=f"out_all_{p}")
    kv_ps = [kvps_pool.tile([M, E], F32, tag="kvps", name=f"kvps_{p}_{i}")
             for i in range(2)]
    kv_sb = [None, None]
    cs64 = lambda c: slice(c * D, (c + 1) * D)
    for c in range(NCH):
        st_ps = ptr.tile([C, C], F32, tag="st", name=f"st_{p}{c}")
        nc.tensor.matmul(st_ps[:], lhsT=KpT_all[:, cs(c)],
                         rhs=QpT_all[:, cs(c)], start=True, stop=True)
        stm = scr.tile([C, C], F16, tag="stm", name=f"stm_{p}{c}")
        nc.vector.tensor_mul(stm[:], st_ps[:], mask_ut[:])

        ops_ps = pops.tile([C, E], F32, tag="ops", name=f"ops_{p}{c}")
        rhs_list = [kv_sb[par] for par in range(2) if kv_sb[par] is not None]
        nc.tensor.matmul(ops_ps[:], lhsT=stm[:], rhs=vaug_sb[:, cs65(c)],
                         start=True, stop=(not rhs_list))
        for n, kvt in enumerate(rhs_list):
            nc.tensor.matmul(ops_ps[:], lhsT=QpT_all[:, cs(c)], rhs=kvt[:],
                             start=False, stop=(n == len(rhs_list) - 1))

        par = c % 2
        nc.tensor.matmul(kv_ps[par][:],
                         lhsT=Kp_all[:, cs(c)], rhs=vaug_sb[:, cs65(c)],
                         start=(c < 2), stop=(c >= NCH - 2),
                         skip_group_check=True)
        if c < NCH - 1:
            kv_sb[par] = kvsb_pool.tile([M, E], F16, tag="kvsb",
                                        name=f"kvsb_{p}_{c}")
            nc.vector.tensor_copy(kv_sb[par][:], kv_ps[par][:])

        rc = small.tile([C, 1], F32, tag="rc", name=f"rc_{p}{c}")
        nc.vector.reciprocal(rc[:], ops_ps[:, D:E])
        nc.scalar.activation(out_all[:, cs64(c)], ops_ps[:, 0:D], AF.Copy,
                             scale=rc[:])

    HO = NCH * D // 2
    nc.sync.dma_start(out[p][:, 0:HO], out_all[:, 0:HO])
    nc.sync.dma_start(out[p][:, HO:2 * HO], out_all[:, HO:2 * HO])


def _kernel(ctx, tc, out, xT, xcm, vaugp, cPTd):
    nc = tc.nc
    const = ctx.enter_context(tc.tile_pool(name="const", bufs=1))
    big = ctx.enter_context(tc.tile_pool(name="big", bufs=2))
    small = ctx.enter_context(tc.tile_pool(name="small", bufs=8))
    scr = ctx.enter_context(tc.tile_pool(name="scr", bufs=6))
    pdash = ctx.enter_context(tc.tile_pool(name="pdash", bufs=2, space="PSUM"))
    ptr = ctx.enter_context(tc.tile_pool(name="ptr", bufs=2, space="PSUM"))
    pops = ctx.enter_context(tc.tile_pool(name="pops", bufs=2, space="PSUM"))
    kvps_pool = ctx.enter_context(tc.tile_pool(name="kvps", bufs=2,
                                               space="PSUM"))
    kvsb_pool = ctx.enter_context(tc.tile_pool(name="kvsb", bufs=8))
    pst = psmall = ptr
    pools = (const, big, small, scr, pdash, ptr, pst, pops, kvps_pool,
             kvsb_pool, psmall)

    identF16 = const.tile([128, 128], F16)
    masks.make_identity(nc, identF16[:])
    mask_ut = const.tile([128, 128], F16)
    masks.make_upper_triangular(nc, mask_ut[:], val=1.0, diag=True)
    ones_row = const.tile([1, 128], F32)
    nc.any.memset(ones_row[:], 1.0)
    cPT = const.tile([2 * D, M], F16)
    nc.sync.dma_start(cPT[:], cPTd[:])
    biasx = const.tile([128, 1], F32)
    nc.any.memset(biasx[:], XBIAS)
    biasA = const.tile([128, 1], F32)
    nc.any.memset(biasA[:], LN_ALPHA)
    biasB = const.tile([128, 1], F32)
    nc.any.memset(biasB[:], LN_BETA)
    consts = (identF16, mask_ut, ones_row, cPT, biasx, biasA, biasB)

    loads = []
    for p in range(PPC):
        loads.append(_emit_load(ctx, tc, pools, p, xT, xcm, vaugp))
        _emit_load2(ctx, tc, loads, p, xT, xcm, vaugp)
    for p in range(PPC):
        kT_sb, kcm_sb, vaug_sb, qT_sb, qcm_sb = loads[p]
        Kp, KpT = _emit_side(ctx, tc, pools, consts, p, "k", kT_sb, kcm_sb)
        _, QpT = _emit_side(ctx, tc, pools, consts, p, "q", qT_sb, qcm_sb)
        _emit_scan(ctx, tc, pools, consts, p, (Kp, KpT, QpT), vaug_sb, out)


def _split_multiwaits(nc):
    """The installed walrus encodes at most ONE semaphore wait per
    instruction (EventSemaphore excepted, which takes two).  Hoist extra
    wait conditions onto preceding EventSemaphores on the same engine —
    pure wait instructions, no pipeline flush."""
    fix_id = [0]

    def wait_ev(engine, waits):
        fix_id[0] += 1
        return mybir.InstEventSemaphore(
            name=f"I-waitfix-{fix_id[0]}",
            opcode="EventSemaphore",
            engine=engine,
            ins=[], outs=[],
            sync_info=mybir.SyncInfo(on_wait=list(waits), on_update=[]),
        )

    for fn in nc.m.functions:
        for blk in fn.blocks:
            new_insts = []
            for inst in blk.instructions:
                si = inst.sync_info
                waits = list(si.on_wait) if si is not None else []
                is_ev = type(inst).__name__ == "InstEventSemaphore"
                cap = 2 if is_ev else 1
                if len(waits) > cap:
                    extra, keep = waits[:-cap], waits[-cap:]
                    for i in range(0, len(extra), 2):
                        new_insts.append(wait_ev(inst.engine, extra[i:i + 2]))
                    si.on_wait = keep
                new_insts.append(inst)
            blk.instructions[:] = new_insts


def _build():
    if 'nc' in _cache:
        return _cache['nc']
    nc = bass.Bass("TRN2", target_bir_lowering=False, debug=False,
                   num_devices=NCORES)
    # xT[p, 0]=kT, xT[p, 1]=qT: [64, L] fp16; xcm likewise chunk-major
    xT = nc.dram_tensor("xT", [PPC, 2, 2 * D, L // 2], F16,
                        kind="ExternalInput").ap()
    xcm = nc.dram_tensor("xcm", [PPC, 2, C, NCH * D], F16,
                         kind="ExternalInput").ap()
    vaugp = nc.dram_tensor("vaugp", [PPC, C, NCH * E], F16,
                           kind="ExternalInput").ap()
    cPTd = nc.dram_tensor("cPTd", [2 * D, M], F16, kind="ExternalInput").ap()
    out = nc.dram_tensor("out", [PPC, C, NCH * D], F16,
                         kind="ExternalOutput").ap()
    with tile.TileContext(nc) as tc:
        with ExitStack() as ctx:
            _kernel(ctx, tc, out, xT, xcm, vaugp, cPTd)
    _split_multiwaits(nc)
    _cache['nc'] = nc
    return nc


def kernel(query, key, value, projection_matrix, _trace=False):
    """Full inputs in, full output out. Shards (b,h) pairs across 8 cores."""
    query = np.asarray(query, dtype=np.float32)
    key = np.asarray(key, dtype=np.float32)
    value = np.asarray(value, dtype=np.float32)
    projection_matrix = np.asarray(projection_matrix, dtype=np.float32)

    nc = _build()

    # [B,L,H,D] -> [B*H, L, D] pair-major
    def pairs_ld(x):
        return np.ascontiguousarray(x.transpose(0, 2, 1, 3).reshape(B * H, L, D))

    # chunk-major [B*H, 128, NCH*D]: row p holds [chunk][d] for position p
    def chunkmaj(x_ld):
        return np.ascontiguousarray(
            x_ld.reshape(B * H, NCH, C, D).transpose(0, 2, 1, 3)
            .reshape(B * H, C, NCH * D))

    q_ld = pairs_ld(query)
    k_ld = pairs_ld(key)
    v_ld = pairs_ld(value)
    # stacked [B*H, 2(k,q), 128, L/2] fp16: partitions 0-63 d x first
    # L-half, 64-127 d x second L-half (halves per-partition DMA bytes)
    xT = np.stack([k_ld.transpose(0, 2, 1), q_ld.transpose(0, 2, 1)], axis=1)
    xT = xT.reshape(B * H, 2, D, 2, L // 2).transpose(0, 1, 3, 2, 4)
    xT = np.ascontiguousarray(
        xT.reshape(B * H, 2, 2 * D, L // 2).astype(np.float16))
    xcm = np.stack([chunkmaj(k_ld), chunkmaj(q_ld)], axis=1)
    xcm = np.ascontiguousarray(xcm.astype(np.float16))
    # V with a baked ones column: [B*H, 128, NCH*(D+1)] fp16
    v4 = v_ld.reshape(B * H, NCH, C, D).transpose(0, 2, 1, 3)
    vaug = np.concatenate(
        [v4, np.ones((B * H, C, NCH, 1), dtype=np.float32)], axis=3)
    vaug = np.ascontiguousarray(
        vaug.reshape(B * H, C, NCH * E).astype(np.float16))
    cPT1 = (DN * projection_matrix).T.astype(np.float16)
    cPT = np.ascontiguousarray(np.concatenate([cPT1, cPT1], axis=0))

    in_maps = []
    for r in range(NCORES):
        sl = slice(r * PPC, (r + 1) * PPC)
        in_maps.append({
            "xT": xT[sl], "xcm": xcm[sl], "vaugp": vaug[sl],
            "cPTd": cPT.copy(),
        })

    res = run_bass_kernel_spmd(nc, in_maps, list(range(NCORES)), trace=_trace)
    out_cm = np.empty((B * H, C, NCH * D), dtype=np.float32)
    for r in range(NCORES):
        out_cm[r * PPC:(r + 1) * PPC] = np.asarray(
            res.results[r]["out"], dtype=np.float32)
    # chunk-major -> [B*H, L, D] -> [B, L, H, D]
    out_ld = out_cm.reshape(B * H, C, NCH, D).transpose(0, 2, 1, 3).reshape(
        B * H, L, D)
    full = out_ld.reshape(B, H, L, D).transpose(0, 2, 1, 3)
    if _trace:
        return np.ascontiguousarray(full), res
    return np.ascontiguousarray(full)


# revision 51
# speedup vs baseline: 1.8313x; 1.0181x over previous
"""FAVOR+ (Performer) causal linear attention on 8 Trainium2 NeuronCores.

Problem: B=2, L=2048, H=8, D=64, M=128 random features, fp32 in/out.
Sharding: the 16 (b,h) pairs are data-parallel; each of the 8 cores gets 2
pairs and runs the full feature-map + chunked causal scan for them with no
cross-core communication.

Math per (b,h) pair (C=128 position chunks, 16 chunks), exactly matching the
reference semantics including the +EPS terms (which are NOT negligible here:
typical k' values are within an order of magnitude of EPS=1e-6):
  q'_t = exp(qdash_t - qdiag_t - qmax_t) + EPS     (per-position stabilizer)
  k'_s = exp(kdash_s - kdiag_s - gmax) + EPS       (global stabilizer)
  out_t = (sum_{s<=t} q'_t.k'_s * v_s) / (sum_{s<=t} q'_t.k'_s)
  (the reference's ratio=1/sqrt(M) cancels in num/den and is dropped)

On-device numerics are fp16 (1 cyc/row matmuls, fp16-2x DVE modes, half the
DMA bytes) with three exact algebraic rescalings that keep every
intermediate inside fp16 range:
  - exp carries a constant bias: Xe = exp(dash - 7.5). It cancels because
    the stabilizers divide by the max of the SAME biased exponentials.
  - q' is scaled by BETA=64 and k' by ALPHA=1024 (EPS scaled along): global
    per-side constants that cancel in num/den and lift the S = k'.q'
    products (~1e-12 in reference units) out of fp16-underflow territory.
Validated vs the fp32 reference: rel-to-scale err ~1.4e-3 (gate 2e-2).

Engine layout (all PSUM-touching elementwise on DVE/Act only -- the BIR
verifier rejects GPSIMD-PSUM access):
  PE:   16 per-chunk dash matmuls per side (shared cPT, fp16) into 1-bank
        PSUM strips; per-chunk fp16 transposes batched 4-to-a-bank; scan
        matmuls (masked S^T, S~V+2 inter, dual-parity KV accumulation).
  Act:  batched exp per [C,512] strip with AP bias, exp(-diag) columns,
        half the transpose drains, out = num*recip(den) via Copy+scale.
  DVE:  segmented-AP reduces (diag sums, per-strip q rowmax), mask multiply
        (PSUM->SBUF fp16), KV snapshots, reciprocals, half of scale+EPS.
  Pool: squares, scale+EPS, per-strip global-max pieces (SBUF only).
  The k/q stabilizer maxes are computed per exp-strip so they pipeline
  behind the remaining dash matmuls instead of serializing after them.
  xT is shipped as [128, L/2] (two d-halves stacked on partitions) to halve
  per-partition DMA bytes; input DMAs are ordered pair-0-first so the
  pair-0 scan overlaps pair-1 prep.
"""

import numpy as np
from contextlib import ExitStack

import concourse.bass as bass
import concourse.mybir as mybir
from concourse import tile, masks
from concourse.bass_utils import run_bass_kernel_spmd

B, L, H, D, M = 2, 2048, 8, 64, 128
C = 128
NCH = L // C              # 16 chunks
E = D + 1                 # 65: value dim + denominator column
NCORES = 8
PPC = (B * H) // NCORES   # 2 (b,h) pairs per core
EPS = 1e-6
DN = 1.0 / (64.0 ** 0.25)       # data_normalizer c
XBIAS = -7.5                    # constant exp bias (cancels via emax/Gexp)
ALPHA = 1024.0                  # k' global rescale (cancels in num/den)
BETA = 64.0                     # q' global rescale (cancels in num/den)
LN_ALPHA = float(np.log(ALPHA))
LN_BETA = float(np.log(BETA))
F32 = mybir.dt.float32
F16 = mybir.dt.float16
AX = mybir.AxisListType
OP = mybir.AluOpType
AF = mybir.ActivationFunctionType

_cache = {}


def _emit_load(ctx, tc, pools, p, xT, xcm, vaugp):
    nc = tc.nc
    (const, big, small, scr, pdash, ptr, pst, pops, kvps_pool, kvsb_pool,
     psmall) = pools
    HL = L // 2
    kT_sb = big.tile([2 * D, HL], F16, tag="kT", name=f"kT_{p}")
    nc.sync.dma_start(kT_sb[:, 0:HL // 2], xT[p, 0][:, 0:HL // 2])
    nc.sync.dma_start(kT_sb[:, HL // 2:HL], xT[p, 0][:, HL // 2:HL])
    kcm_sb = big.tile([C, NCH * D], F16, tag="kcm", name=f"kcm_{p}")
    nc.sync.dma_start(kcm_sb[:], xcm[p, 0])
    qT_sb = big.tile([2 * D, HL], F16, tag="qT", name=f"qT_{p}")
    qcm_sb = big.tile([C, NCH * D], F16, tag="qcm", name=f"qcm_{p}")
    vaug_sb = big.tile([C, NCH * E], F16, tag="vaug", name=f"vaug_{p}")
    return kT_sb, kcm_sb, vaug_sb, qT_sb, qcm_sb


def _emit_load2(ctx, tc, loads, p, xT, xcm, vaugp):
    nc = tc.nc
    kT_sb, kcm_sb, vaug_sb, qT_sb, qcm_sb = loads[p]
    nc.sync.dma_start(qT_sb[:], xT[p, 1])
    nc.sync.dma_start(qcm_sb[:], xcm[p, 1])
    nc.sync.dma_start(vaug_sb[:], vaugp[p])


def _emit_side(ctx, tc, pools, consts, p, side, xT_sb, xcm_sb):
    """Feature map for one side (k or q) of one pair.

    Returns (Xp_all [C, NCH*M] fp16 position-major primed features or None
    for q, XpT_all [M, L] fp16 feature-major primed features)."""
    nc = tc.nc
    identF16, mask_ut, ones_row, cPT, biasx, biasA, biasB = consts
    (const, big, small, scr, pdash, ptr, pst, pops, kvps_pool, kvsb_pool,
     psmall) = pools
    is_k = side == "k"
    cs = lambda c: slice(c * C, (c + 1) * C)

    # dash: per-chunk matmuls into 1-bank PSUM strips (4 chunks each),
    # shared moving cPT; one batched biased exp per strip
    Xe_all = big.tile([C, NCH * M], F16, tag=f"{side}e", name=f"{side}e_{p}")
    if is_k:
        gmp = small.tile([1, NCH // 4], F32, tag="gmp", name=f"gmp_{p}")
    else:
        emax = small.tile([C, NCH], F16, tag="emax", name=f"emax_{p}")
    for g in range(NCH // 4):
        xd_ps = pdash.tile([C, 4 * C], F32, tag="dash",
                           name=f"dash_{p}{side}{g}")
        for i in range(4):
            c = 4 * g + i
            if c < NCH // 2:
                lhsT, rhs = xT_sb[0:D, cs(c)], cPT[0:D, :]
            else:
                lhsT, rhs = xT_sb[D:2 * D, cs(c - NCH // 2)], cPT[D:2 * D, :]
            nc.tensor.matmul(xd_ps[:, i * C:(i + 1) * C],
                             lhsT=lhsT, rhs=rhs,
                             start=True, stop=True)
        strip = Xe_all[:, g * 4 * C:(g + 1) * 4 * C]
        nc.scalar.activation(strip, xd_ps[:], AF.Exp, bias=biasx[:])
        # per-strip stabilizer reduce, pipelined behind the next strip
        if is_k:
            nc.gpsimd.tensor_reduce(gmp[:, g:g + 1], strip,
                                    axis=AX.XYZWC, op=OP.max)
        else:
            nc.vector.tensor_reduce(
                emax[:, 4 * g:4 * (g + 1)],
                strip.rearrange("p (c m) -> p c m", c=4),
                axis=AX.X, op=OP.max)

    # diag column: square chunk-major x (Pool), segmented add-reduce (DVE)
    xsq = scr.tile([C, NCH * D], F16, tag="xsq", name=f"xsq_{p}{side}")
    nc.gpsimd.tensor_mul(xsq[:], xcm_sb[:], xcm_sb[:])
    xdiag = small.tile([C, NCH], F16, tag="xdiag", name=f"xdiag_{p}{side}")
    with nc.allow_low_precision(reason="sum of 64 fp16 squares; validated "
                                "diag abs err ~2e-3 -> 0.2% weight error"):
        nc.vector.tensor_reduce(xdiag[:],
                                xsq[:].rearrange("p (c d) -> p c d", c=NCH),
                                axis=AX.X, op=OP.add)
    # endx = SCALE * exp(-c^2/2 * sum x^2): the 0.0625 folds c^2/2
    endx = small.tile([C, NCH], F32, tag="endx", name=f"endx_{p}{side}")
    nc.scalar.activation(endx[:], xdiag[:], AF.Exp, scale=-0.0625,
                         bias=(biasA[:] if is_k else biasB[:]))

    bcol = small.tile([C, NCH], F32, tag="bcol", name=f"bcol_{p}{side}")
    if is_k:
        # global stabilizer: 1/max over ALL (position, m) of the biased exp
        gm = small.tile([1, 1], F32, tag="gm", name=f"gm_{p}")
        nc.vector.tensor_reduce(gm[:], gmp[:], axis=AX.X, op=OP.max)
        rg = small.tile([1, 1], F32, tag="rg", name=f"rg_{p}")
        nc.vector.reciprocal(rg[:], gm[:])
        nsb = ptr.tile([C, 1], F32, tag="st", name=f"nsb_{p}")
        nc.tensor.matmul(nsb[:], lhsT=ones_row[:], rhs=rg[:], start=True,
                         stop=True)
        rgb = small.tile([C, 1], F32, tag="rgb", name=f"rgb_{p}")
        nc.vector.tensor_copy(rgb[:], nsb[:])
        nc.gpsimd.tensor_scalar_mul(bcol[:], endx[:], rgb[:])
    else:
        # per-position stabilizer: exp(-diag)/rowmax(exp)
        remax = small.tile([C, NCH], F32, tag="remax", name=f"remax_{p}")
        nc.vector.reciprocal(remax[:], emax[:])
        nc.gpsimd.tensor_mul(bcol[:], endx[:], remax[:])

    # x' = exp * bcol + EPS' (Pool), then PE-transpose each chunk to
    # feature-major; 4 transposed chunks batch into one f16 PSUM tile so a
    # single fp16-2x copy drains them (alternating DVE / Act)
    eps_s = ALPHA * EPS if is_k else BETA * EPS
    XpT_all = big.tile([M, L], F16, tag=f"{side}pT", name=f"{side}pT_{p}")
    Xp_all = big.tile([C, NCH * M], F16, tag=f"{side}p", name=f"{side}p_{p}")
    for g in range(NCH // 4):
        tp4 = pdash.tile([M, 4 * C], F16, tag="dash", name=f"tp4_{p}{side}{g}")
        for i in range(4):
            c = 4 * g + i
            nc.gpsimd.tensor_scalar(out=Xp_all[:, cs(c)],
                                    in0=Xe_all[:, cs(c)],
                                    scalar1=bcol[:, c:c + 1], scalar2=eps_s,
                                    op0=OP.mult, op1=OP.add)
            nc.tensor.transpose(tp4[:, i * C:(i + 1) * C], Xp_all[:, cs(c)],
                                identF16[:])
        dst = XpT_all[:, g * 4 * C:(g + 1) * 4 * C]
        if g % 2 == 0:
            nc.vector.tensor_copy(dst, tp4[:])
        else:
            nc.scalar.activation(dst, tp4[:], AF.Copy)
    return (Xp_all if is_k else None), XpT_all


def _emit_scan(ctx, tc, pools, consts, p, state, vaug_sb, out):
    nc = tc.nc
    identF16, mask_ut, ones_row, cPT, biasx, biasA, biasB = consts
    (const, big, small, scr, pdash, ptr, pst, pops, kvps_pool, kvsb_pool,
     psmall) = pools
    Kp_all, KpT_all, QpT_all = state

    cs = lambda c: slice(c * C, (c + 1) * C)
    cs65 = lambda c: slice(c * E, (c + 1) * E)

    out_all = big.tile([C, NCH * D], F16, tag="out_all", name=f"out_all_{p}")
    kv_ps = [kvps_pool.tile([M, E], F32, tag="kvps", name=f"kvps_{p}_{i}")
             for i in range(2)]
    kv_sb = [None, None]
    cs64 = lambda c: slice(c * D, (c + 1) * D)
    for c in range(NCH):
        st_ps = ptr.tile([C, C], F32, tag="st", name=f"st_{p}{c}")
        nc.tensor.matmul(st_ps[:], lhsT=KpT_all[:, cs(c)],
                         rhs=QpT_all[:, cs(c)], start=True, stop=True)
        stm = scr.tile([C, C], F16, tag="stm", name=f"stm_{p}{c}")
        nc.vector.tensor_mul(stm[:], st_ps[:], mask_ut[:])

        ops_ps = pops.tile([C, E], F32, tag="ops", name=f"ops_{p}{c}")
        rhs_list = [kv_sb[par] for par in range(2) if kv_sb[par] is not None]
        nc.tensor.matmul(ops_ps[:], lhsT=stm[:], rhs=vaug_sb[:, cs65(c)],
                         start=True, stop=(not rhs_list))
        for n, kvt in enumerate(rhs_list):
            nc.tensor.matmul(ops_ps[:], lhsT=QpT_all[:, cs(c)], rhs=kvt[:],
                             start=False, stop=(n == len(rhs_list) - 1))

        par = c % 2
        nc.tensor.matmul(kv_ps[par][:],
                         lhsT=Kp_all[:, cs(c)], rhs=vaug_sb[:, cs65(c)],
                         start=(c < 2), stop=(c >= NCH - 2),
                         skip_group_check=True)
        if c < NCH - 1:
            kv_sb[par] = kvsb_pool.tile([M, E], F16, tag="kvsb",
                                        name=f"kvsb_{p}_{c}")
            if c % 8 == 7:
                nc.scalar.activation(kv_sb[par][:], kv_ps[par][:], AF.Copy)
            else:
                nc.vector.tensor_copy(kv_sb[par][:], kv_ps[par][:])

        rc = small.tile([C, 1], F32, tag="rc", name=f"rc_{p}{c}")
        nc.vector.reciprocal(rc[:], ops_ps[:, D:E])
        nc.scalar.activation(out_all[:, cs64(c)], ops_ps[:, 0:D], AF.Copy,
                             scale=rc[:])

    HO = NCH * D // 2
    nc.sync.dma_start(out[p][:, 0:HO], out_all[:, 0:HO])
    nc.sync.dma_start(out[p][:, HO:2 * HO], out_all[:, HO:2 * HO])


def _kernel(ctx, tc, out, xT, xcm, vaugp, cPTd):
    nc = tc.nc
    const = ctx.enter_context(tc.tile_pool(name="const", bufs=1))
    big = ctx.enter_context(tc.tile_pool(name="big", bufs=2))
    small = ctx.enter_context(tc.tile_pool(name="small", bufs=8))
    scr = ctx.enter_context(tc.tile_pool(name="scr", bufs=6))
    pdash = ctx.enter_context(tc.tile_pool(name="pdash", bufs=2, space="PSUM"))
    ptr = ctx.enter_context(tc.tile_pool(name="ptr", bufs=2, space="PSUM"))
    pops = ctx.enter_context(tc.tile_pool(name="pops", bufs=2, space="PSUM"))
    kvps_pool = ctx.enter_context(tc.tile_pool(name="kvps", bufs=2,
                                               space="PSUM"))
    kvsb_pool = ctx.enter_context(tc.tile_pool(name="kvsb", bufs=8))
    pst = psmall = ptr
    pools = (const, big, small, scr, pdash, ptr, pst, pops, kvps_pool,
             kvsb_pool, psmall)

    identF16 = const.tile([128, 128], F16)
    masks.make_identity(nc, identF16[:])
    mask_ut = const.tile([128, 128], F16)
    masks.make_upper_triangular(nc, mask_ut[:], val=1.0, diag=True)
    ones_row = const.tile([1, 128], F32)
    nc.any.memset(ones_row[:], 1.0)
    cPT = const.tile([2 * D, M], F16)
    nc.sync.dma_start(cPT[:], cPTd[:])
    biasx = const.tile([128, 1], F32)
    nc.any.memset(biasx[:], XBIAS)
    biasA = const.tile([128, 1], F32)
    nc.any.memset(biasA[:], LN_ALPHA)
    biasB = const.tile([128, 1], F32)
    nc.any.memset(biasB[:], LN_BETA)
    consts = (identF16, mask_ut, ones_row, cPT, biasx, biasA, biasB)

    loads = []
    for p in range(PPC):
        loads.append(_emit_load(ctx, tc, pools, p, xT, xcm, vaugp))
        _emit_load2(ctx, tc, loads, p, xT, xcm, vaugp)
    for p in range(PPC):
        kT_sb, kcm_sb, vaug_sb, qT_sb, qcm_sb = loads[p]
        Kp, KpT = _emit_side(ctx, tc, pools, consts, p, "k", kT_sb, kcm_sb)
        _, QpT = _emit_side(ctx, tc, pools, consts, p, "q", qT_sb, qcm_sb)
        _emit_scan(ctx, tc, pools, consts, p, (Kp, KpT, QpT), vaug_sb, out)


def _split_multiwaits(nc):
    """The installed walrus encodes at most ONE semaphore wait per
    instruction (EventSemaphore excepted, which takes two).  Hoist extra
    wait conditions onto preceding EventSemaphores on the same engine —
    pure wait instructions, no pipeline flush."""
    fix_id = [0]

    def wait_ev(engine, waits):
        fix_id[0] += 1
        return mybir.InstEventSemaphore(
            name=f"I-waitfix-{fix_id[0]}",
            opcode="EventSemaphore",
            engine=engine,
            ins=[], outs=[],
            sync_info=mybir.SyncInfo(on_wait=list(waits), on_update=[]),
        )

    for fn in nc.m.functions:
        for blk in fn.blocks:
            new_insts = []
            for inst in blk.instructions:
                si = inst.sync_info
                waits = list(si.on_wait) if si is not None else []
                is_ev = type(inst).__name__ == "InstEventSemaphore"
                cap = 2 if is_ev else 1
                if len(waits) > cap:
                    extra, keep = waits[:-cap], waits[-cap:]
                    for i in range(0, len(extra), 2):
                        new_insts.append(wait_ev(inst.engine, extra[i:i + 2]))
                    si.on_wait = keep
                new_insts.append(inst)
            blk.instructions[:] = new_insts


def _build():
    if 'nc' in _cache:
        return _cache['nc']
    nc = bass.Bass("TRN2", target_bir_lowering=False, debug=False,
                   num_devices=NCORES)
    # xT[p, 0]=kT, xT[p, 1]=qT: [64, L] fp16; xcm likewise chunk-major
    xT = nc.dram_tensor("xT", [PPC, 2, 2 * D, L // 2], F16,
                        kind="ExternalInput").ap()
    xcm = nc.dram_tensor("xcm", [PPC, 2, C, NCH * D], F16,
                         kind="ExternalInput").ap()
    vaugp = nc.dram_tensor("vaugp", [PPC, C, NCH * E], F16,
                           kind="ExternalInput").ap()
    cPTd = nc.dram_tensor("cPTd", [2 * D, M], F16, kind="ExternalInput").ap()
    out = nc.dram_tensor("out", [PPC, C, NCH * D], F16,
                         kind="ExternalOutput").ap()
    with tile.TileContext(nc) as tc:
        with ExitStack() as ctx:
            _kernel(ctx, tc, out, xT, xcm, vaugp, cPTd)
    _split_multiwaits(nc)
    _cache['nc'] = nc
    return nc


def kernel(query, key, value, projection_matrix, _trace=False):
    """Full inputs in, full output out. Shards (b,h) pairs across 8 cores."""
    query = np.asarray(query, dtype=np.float32)
    key = np.asarray(key, dtype=np.float32)
    value = np.asarray(value, dtype=np.float32)
    projection_matrix = np.asarray(projection_matrix, dtype=np.float32)

    nc = _build()

    # [B,L,H,D] -> [B*H, L, D] pair-major
    def pairs_ld(x):
        return np.ascontiguousarray(x.transpose(0, 2, 1, 3).reshape(B * H, L, D))

    # chunk-major [B*H, 128, NCH*D]: row p holds [chunk][d] for position p
    def chunkmaj(x_ld):
        return np.ascontiguousarray(
            x_ld.reshape(B * H, NCH, C, D).transpose(0, 2, 1, 3)
            .reshape(B * H, C, NCH * D))

    q_ld = pairs_ld(query)
    k_ld = pairs_ld(key)
    v_ld = pairs_ld(value)
    # stacked [B*H, 2(k,q), 128, L/2] fp16: partitions 0-63 d x first
    # L-half, 64-127 d x second L-half (halves per-partition DMA bytes)
    xT = np.stack([k_ld.transpose(0, 2, 1), q_ld.transpose(0, 2, 1)], axis=1)
    xT = xT.reshape(B * H, 2, D, 2, L // 2).transpose(0, 1, 3, 2, 4)
    xT = np.ascontiguousarray(
        xT.reshape(B * H, 2, 2 * D, L // 2).astype(np.float16))
    xcm = np.stack([chunkmaj(k_ld), chunkmaj(q_ld)], axis=1)
    xcm = np.ascontiguousarray(xcm.astype(np.float16))
    # V with a baked ones column: [B*H, 128, NCH*(D+1)] fp16
    v4 = v_ld.reshape(B * H, NCH, C, D).transpose(0, 2, 1, 3)
    vaug = np.concatenate(
        [v4, np.ones((B * H, C, NCH, 1), dtype=np.float32)], axis=3)
    vaug = np.ascontiguousarray(
        vaug.reshape(B * H, C, NCH * E).astype(np.float16))
    cPT1 = (DN * projection_matrix).T.astype(np.float16)
    cPT = np.ascontiguousarray(np.concatenate([cPT1, cPT1], axis=0))

    in_maps = []
    for r in range(NCORES):
        sl = slice(r * PPC, (r + 1) * PPC)
        in_maps.append({
            "xT": xT[sl], "xcm": xcm[sl], "vaugp": vaug[sl],
            "cPTd": cPT.copy(),
        })

    res = run_bass_kernel_spmd(nc, in_maps, list(range(NCORES)), trace=_trace)
    out_cm = np.empty((B * H, C, NCH * D), dtype=np.float32)
    for r in range(NCORES):
        out_cm[r * PPC:(r + 1) * PPC] = np.asarray(
            res.results[r]["out"], dtype=np.float32)
    # chunk-major -> [B*H, L, D] -> [B, L, H, D]
    out_ld = out_cm.reshape(B * H, C, NCH, D).transpose(0, 2, 1, 3).reshape(
        B * H, L, D)
    full = out_ld.reshape(B, H, L, D).transpose(0, 2, 1, 3)
    if _trace:
        return np.ascontiguousarray(full), res
    return np.ascontiguousarray(full)
